# revision 24
# baseline (speedup 1.0000x reference)
"""Host-side sharding/prep + Bass device program for nn_BBGRUDecoder.

Host does index manipulation / data layout only; the device kernel does all
model arithmetic.

v4 design:
- conv1 slot arrays carry the root feature as slot KSLOT (weight 1.0) and are
  shipped pre-transposed [SLOT_W, rows] so the conv1 matmul needs no
  on-device transpose and no tree-reduce.
- conv2 does NOT gather h1 rows (SWDGE gather costs ~10ns/row on gpsimd).
  Instead the host lays out each edge's SOURCE-node slot data edge-major
  ([SLOT_W, E2_slots]) and the device recomputes h1 per edge subtile with one
  extra matmul+relu. No gather, no compaction, no DRAM h1 tables.
- conv1 computes h1 only for local V tiles (root term), kept resident in SBUF.
- s2 (edge->dst scatter weights) shipped transposed [128, E2_slots] for wide
  contiguous loads; s3/pool_gid/inv_cnt/emb are single-DMA preloads.
"""
import numpy as np
import ml_dtypes

BF16 = np.dtype(ml_dtypes.bfloat16)
NC = 8
P = 128
KSLOT = 16       # conv1 in-edge slots per node (max in-degree 13)
KSLOT2 = 17      # + root slot
F_IN = 5
SLOT_W = KSLOT2 * F_IN   # 85
F1 = 128
F2 = 256
HID = 128
TR = 10          # rounds per shot
MAXG_TILE = 32   # max graphs per node-tile (pool S3 width)
OCT = 16         # conv1 tiles per input DMA


def _pack_groups(sizes, cap_items, cap_groups):
    """Greedy-pack consecutive groups (each <=cap_items items) into tiles of
    <=cap_items items and <=cap_groups groups."""
    tiles = []
    i = 0
    n = len(sizes)
    while i < n:
        items = 0
        g = 0
        while i + g < n and g < cap_groups and items + sizes[i + g] <= cap_items:
            items += sizes[i + g]
            g += 1
        assert g > 0, f"group {i} size {sizes[i]} exceeds cap {cap_items}"
        tiles.append((i, g, items))
        i += g
    return tiles


def prep(inputs):
    x = np.asarray(inputs["x"], np.float32)
    ei = np.asarray(inputs["edge_index"], np.int64)
    ea = np.asarray(inputs["edge_attr"], np.float32)
    bl = np.asarray(inputs["batch_labels"], np.int64)
    lm = np.asarray(inputs["label_map"], np.int64)
    B = int(inputs["B"])
    NN = x.shape[0]
    src_g, dst_g = ei[0], ei[1]
    shot_of, round_of = lm[:, 0], lm[:, 1]
    n_shot_core = (B + NC - 1) // NC          # 128 shots per core
    GCOLS = n_shot_core * TR                  # 1280 graph-columns per core
    deg = np.bincount(dst_g, minlength=NN)
    assert deg.max() <= KSLOT

    # ---- global conv1 slot data [NN, KSLOT2, F_IN]; slot KSLOT = root ----
    xs_all = np.zeros((NN, KSLOT2, F_IN), np.float32)
    ea_all = np.zeros((NN, KSLOT2, F_IN), np.float32)
    xs_all[:, KSLOT] = x
    ea_all[:, KSLOT] = 1.0
    order = np.argsort(dst_g, kind="stable")
    ds = dst_g[order]
    sl = np.arange(len(ds)) - np.searchsorted(ds, ds)   # slot within dst run
    xs_all[ds, sl] = x[src_g[order]]
    ea_all[ds, sl] = ea[order][:, None]
    xs_flat = xs_all.reshape(NN, SLOT_W).astype(BF16)
    ea_flat = ea_all.reshape(NN, SLOT_W).astype(BF16)

    node_g = bl
    node_core = (shot_of[node_g] % NC).astype(np.int64)

    cores = []
    for d in range(NC):
        V = np.nonzero(node_core == d)[0]          # ascending node ids
        gids, gstart, gcnt = np.unique(node_g[V], return_index=True, return_counts=True)
        s_idx = shot_of[gids] // NC
        gcol = s_idx * TR + round_of[gids]
        tiles = _pack_groups(gcnt.tolist(), P, MAXG_TILE)
        NT_V = len(tiles)
        vpos = np.full(NN, -1, np.int64)
        packed_rows = []
        tile_graphs = []
        for (g0, ng, ni) in tiles:
            rows = []
            for k in range(g0, g0 + ng):
                rows.append(V[gstart[k]:gstart[k] + gcnt[k]])
            rows = np.concatenate(rows)
            packed_rows.append(rows)
            tile_graphs.append((gcol[g0:g0 + ng], gcnt[g0:g0 + ng]))
        for t, rows in enumerate(packed_rows):
            vpos[rows] = t * P + np.arange(len(rows))

        E = np.nonzero(node_core[dst_g] == d)[0]
        cores.append(dict(
            d=d, V=V, NT_V=NT_V, packed_rows=packed_rows,
            tile_graphs=tile_graphs, vpos=vpos,
            e_src=src_g[E], e_dst=dst_g[E], e_ea=ea[E],
            gids=gids, gcol=gcol, gcnt=gcnt,
        ))

    # ---- shared static shapes ----
    NT_V = max(c["NT_V"] for c in cores)
    V_pad = NT_V * P

    T_sub = np.zeros(NT_V, np.int64)
    for c in cores:
        for t in range(NT_V):
            if t < c["NT_V"]:
                ne = int(deg[c["packed_rows"][t]].sum())
            else:
                ne = 0
            T_sub[t] = max(T_sub[t], -(-ne // P) if ne else 1)
    E2_slots = int(T_sub.sum()) * P
    NW = -(-NT_V // 4)     # scatter windows (4 tiles each)

    meta = dict(NT_V=NT_V, V_pad=V_pad, T_sub=T_sub.tolist(),
                E2_slots=E2_slots, GCOLS=GCOLS, G_rows=-(-(GCOLS + 1) // P) * P,
                n_shot_core=n_shot_core, B=B, NW=NW)

    # ---- per-core padded arrays ----
    for c in cores:
        vpos = c["vpos"]
        # conv1 V slot data, transposed [SLOT_W, V_pad]
        xsV = np.zeros((V_pad, SLOT_W), BF16)
        eaV = np.zeros((V_pad, SLOT_W), BF16)
        for t, rows in enumerate(c["packed_rows"]):
            xsV[t * P:t * P + len(rows)] = xs_flat[rows]
            eaV[t * P:t * P + len(rows)] = ea_flat[rows]
        xsTV = np.ascontiguousarray(xsV.T)
        eaTV = np.ascontiguousarray(eaV.T)

        # conv2: edge-major src slot data + s2 scatter weights, per tile
        xs2 = np.zeros((E2_slots, SLOT_W), BF16)
        ea2 = np.zeros((E2_slots, SLOT_W), BF16)
        s2 = np.zeros((E2_slots // P, P, P), np.float32)
        st = 0
        for t in range(NT_V):
            nsub = int(T_sub[t])
            if t < c["NT_V"]:
                sel = np.nonzero((vpos[c["e_dst"]] >= t * P) &
                                 (vpos[c["e_dst"]] < t * P + P))[0]
                es, ed, ew = c["e_src"][sel], c["e_dst"][sel], c["e_ea"][sel]
                ne = len(es)
                xs2[st * P:st * P + ne] = xs_flat[es]
                ea2[st * P:st * P + ne] = ea_flat[es]
                loc = vpos[ed] - t * P
                s2[st + np.arange(ne) // P, np.arange(ne) % P, loc] = ew
            st += nsub
        assert st * P == E2_slots
        xs2T = np.ascontiguousarray(xs2.T)
        ea2T = np.ascontiguousarray(ea2.T)
        s2T = np.ascontiguousarray(
            s2.transpose(1, 0, 2).reshape(P, E2_slots).astype(BF16))

        # pool S3 / graph ids / inv counts
        s3 = np.zeros((NT_V, P, MAXG_TILE), np.float32)
        pool_gid = np.full((NT_V, MAXG_TILE), meta["GCOLS"] + 100, np.int64)
        for t in range(c["NT_V"]):
            gcols, gcnts = c["tile_graphs"][t]
            off = 0
            for j, (gc, n) in enumerate(zip(gcols, gcnts)):
                s3[t, off:off + n, j] = 1
                pool_gid[t, j] = gc
                off += n
        s3T = np.ascontiguousarray(s3.transpose(1, 0, 2).astype(BF16))
        pg_pad = np.full((NW * 4, MAXG_TILE), meta["GCOLS"] + 100, np.int64)
        pg_pad[:NT_V] = pool_gid
        pgT = np.ascontiguousarray(pg_pad.reshape(NW, P).T.astype(np.int32))
        inv_cnt = np.zeros(meta["G_rows"], np.float32)
        inv_cnt[c["gcol"]] = 1.0 / np.maximum(c["gcnt"], 1)
        invT = np.ascontiguousarray(inv_cnt.reshape(-1, P).T)

        amask = np.zeros(n_shot_core, np.float32)
        amask[(shot_of[c["gids"]] // NC)] = 1.0

        c["arrays"] = dict(
            xsTV=xsTV, eaTV=eaTV, xs2T=xs2T, ea2T=ea2T, s2T=s2T,
            s3T=s3T, pgT=pgT, invT=invT, amask=amask,
        )
    return cores, meta


# ======================================================
"""Bass/Tile device program (per-core SPMD)."""
import concourse.bass as bass
import concourse.bacc as bacc
import concourse.mybir as mybir
from concourse.tile import TileContext


BF = mybir.dt.bfloat16
FP = mybir.dt.float32
AF = mybir.ActivationFunctionType


def build(meta, num_devices=8):
    NT_V = meta["NT_V"]
    V_pad = meta["V_pad"]
    T_sub = meta["T_sub"]
    E2_slots = meta["E2_slots"]
    GCOLS, G_rows = meta["GCOLS"], meta["G_rows"]
    NSH = meta["n_shot_core"]
    NW = meta["NW"]
    NT_G = G_rows // P

    nc = bacc.Bacc("TRN2", target_bir_lowering=False, debug=False,
                   num_devices=num_devices)

    def inp(name, shape, dt):
        return nc.dram_tensor(name, shape, dt, kind="ExternalInput")

    xsTV_d = inp("xsTV", [SLOT_W, V_pad], BF)
    eaTV_d = inp("eaTV", [SLOT_W, V_pad], BF)
    xs2T_d = inp("xs2T", [SLOT_W, E2_slots], BF)
    ea2T_d = inp("ea2T", [SLOT_W, E2_slots], BF)
    w1s_d = inp("w1s", [P, F1], BF)
    s2T_d = inp("s2T", [P, E2_slots], BF)
    s3T_d = inp("s3T", [P, NT_V, MAXG_TILE], BF)
    pgT_d = inp("pgT", [P, NW], mybir.dt.int32)
    invT_d = inp("invT", [P, NT_G], FP)
    amask_d = inp("amask", [12, NSH], FP)
    ident_d = inp("ident", [P, P], BF)
    wrel2_d = inp("wrel2", [P, F2], BF)
    wroot2_d = inp("wroot2", [P, F2], BF)
    wih0_d = inp("wih0", [3, 2, P, P], BF)
    whh0_d = inp("whh0", [3, P, P], BF)
    wih1_d = inp("wih1", [3, P, P], BF)
    whh1_d = inp("whh1", [3, P, P], BF)
    dec_d = inp("dec", [P, 12], BF)
    out_d = nc.dram_tensor("out", [12, NSH], FP, kind="ExternalOutput")

    emb_d = nc.dram_tensor("emb", [G_rows, F2], FP, kind="Internal")

    with TileContext(nc) as tc:
        with (
            tc.tile_pool(name="const", bufs=1) as cpool,
            tc.tile_pool(name="sb", bufs=3) as pool,
            tc.tile_pool(name="big", bufs=3) as bigp,
            tc.tile_pool(name="psA", bufs=3, space="PSUM") as psA,
            tc.tile_pool(name="psH", bufs=2, space="PSUM") as psH,
            tc.tile_pool(name="psP", bufs=1, space="PSUM") as psP,
            tc.tile_pool(name="psC", bufs=2, space="PSUM") as psC,
        ):
            # ---------------- constants / preloads ----------------
            ident = cpool.tile([P, P], BF, tag="ident")
            nc.sync.dma_start(out=ident[:], in_=ident_d[:])
            w1s = cpool.tile([P, F1], BF, tag="w1s")
            nc.sync.dma_start(out=w1s[:], in_=w1s_d[:])
            wrel2 = cpool.tile([P, F2], BF, tag="wrel2")
            nc.sync.dma_start(out=wrel2[:], in_=wrel2_d[:])
            wroot2 = cpool.tile([P, F2], BF, tag="wroot2")
            nc.sync.dma_start(out=wroot2[:], in_=wroot2_d[:])

            wih0 = []
            for gate in range(3):
                for k in range(2):
                    wt = cpool.tile([P, P], BF, tag=f"wih0_{gate}_{k}")
                    nc.sync.dma_start(out=wt[:], in_=wih0_d[gate, k])
                    wih0.append(wt)

            def load3(dram, nm):
                ts = []
                for i in range(3):
                    wt = cpool.tile([P, P], BF, tag=f"{nm}{i}")
                    nc.sync.dma_start(out=wt[:], in_=dram[i])
                    ts.append(wt)
                return ts

            whh0 = load3(whh0_d, "whh0")
            wih1 = load3(wih1_d, "wih1")
            whh1 = load3(whh1_d, "whh1")
            dec = cpool.tile([P, 12], BF, tag="dec")
            nc.sync.dma_start(out=dec[:], in_=dec_d[:])
            am = cpool.tile([12, NSH], FP, tag="am")
            nc.sync.dma_start(out=am[:], in_=amask_d[:])
            s3all = cpool.tile([P, NT_V, MAXG_TILE], BF, tag="s3all")
            nc.sync.dma_start(out=s3all[:], in_=s3T_d[:])
            pgall = cpool.tile([P, NW], mybir.dt.int32, tag="pgall")
            nc.sync.dma_start(out=pgall[:], in_=pgT_d[:])
            invall = cpool.tile([P, NT_G], FP, tag="invall")
            nc.sync.dma_start(out=invall[:], in_=invT_d[:])

            # zero emb via gpsimd queue so the later indirect scatters
            # (same SWDGE FIFO) are ordered after it without a barrier
            zt = cpool.tile([P, F2], FP, tag="zero")
            nc.gpsimd.memset(zt[:], 0.0)
            for t in range(NT_G):
                nc.gpsimd.dma_start(out=emb_d[t * P:(t + 1) * P, :], in_=zt[:])

            # h1^T of V tiles stays resident for conv2's root term
            # (features on partitions, node columns)
            h1TVall = cpool.tile([P, V_pad], BF, tag="h1TVall")

            # ---------------- conv1 over V tiles ----------------
            n_oct = (NT_V + OCT - 1) // OCT
            for o in range(n_oct):
                t0 = o * OCT
                nt = min(OCT, NT_V - t0)
                cols = slice(t0 * P, (t0 + nt) * P)
                xs_t = pool.tile([SLOT_W, OCT * P], BF, tag="xs1")
                nc.sync.dma_start(out=xs_t[:, :nt * P], in_=xsTV_d[:, cols])
                ea_t = pool.tile([SLOT_W, OCT * P], BF, tag="ea1")
                nc.sync.dma_start(out=ea_t[:, :nt * P], in_=eaTV_d[:, cols])
                msgT = pool.tile([SLOT_W, OCT * P], BF, tag="msg1")
                nc.vector.tensor_mul(out=msgT[:, :nt * P], in0=xs_t[:, :nt * P],
                                     in1=ea_t[:, :nt * P])
                for g0 in range(0, nt, 4):
                    tg = t0 + g0
                    ng = min(4, nt - g0)
                    h1p = psC.tile([P, 4 * P], FP, tag="pC")
                    nc.tensor.matmul(
                        h1p[:, :ng * P], lhsT=w1s[:SLOT_W, :],
                        rhs=msgT[:, g0 * P:(g0 + ng) * P],
                        start=True, stop=True)
                    dst = h1TVall[:, tg * P:(tg + ng) * P]
                    if (o + g0 // 4) % 2 == 0:
                        nc.scalar.activation(dst, h1p[:, :ng * P], AF.Relu)
                    else:
                        nc.vector.tensor_relu(out=dst, in_=h1p[:, :ng * P])

            # ---------------- conv2 + pool (gather-free) ----------------
            GG = 16
            sub_start = np.concatenate([[0], np.cumsum(T_sub)]).astype(int)
            plan = []
            t = 0
            while t < NT_V:
                te = t
                while te < NT_V and sub_start[te + 1] - sub_start[t] <= GG:
                    te += 1
                plan.append((t, te))
                t = te
            pool_ps = None
            for (ta, te) in plan:
                so0 = int(sub_start[ta])
                ns = int(sub_start[te]) - so0
                ecols = slice(so0 * P, (so0 + ns) * P)
                xs2_t = pool.tile([SLOT_W, GG * P], BF, tag="xs2")
                nc.sync.dma_start(out=xs2_t[:, :ns * P], in_=xs2T_d[:, ecols])
                ea2_t = pool.tile([SLOT_W, GG * P], BF, tag="ea2")
                nc.sync.dma_start(out=ea2_t[:, :ns * P], in_=ea2T_d[:, ecols])
                msg2 = pool.tile([SLOT_W, GG * P], BF, tag="msg2")
                nc.vector.tensor_mul(out=msg2[:, :ns * P], in0=xs2_t[:, :ns * P],
                                     in1=ea2_t[:, :ns * P])
                s2g = bigp.tile([P, GG, P], BF, tag="s2g")
                nc.sync.dma_start(
                    out=s2g[:, :ns, :],
                    in_=s2T_d[:, ecols].rearrange("p (s q) -> p s q", q=P))
                # h1 of edge sources, 4 subtiles per PSUM bank
                gts = bigp.tile([P, GG * F1], BF, tag="gts")
                for sb in range(0, ns, 4):
                    nb = min(4, ns - sb)
                    hep = psC.tile([P, 4 * P], FP, tag="pC")
                    for k in range(nb):
                        nc.tensor.matmul(
                            hep[:, k * P:(k + 1) * P],
                            lhsT=msg2[:, (sb + k) * P:(sb + k + 1) * P],
                            rhs=w1s[:SLOT_W, :],
                            start=True, stop=True)
                    dst = gts[:, sb * F1:(sb + nb) * F1]
                    if (sb // 4) % 2 == 0:
                        nc.scalar.activation(dst, hep[:, :nb * P], AF.Relu)
                    else:
                        nc.vector.tensor_relu(out=dst, in_=hep[:, :nb * P])
                for t in range(ta, te):
                    so = int(sub_start[t]) - so0
                    nsub = T_sub[t]
                    agg2T = psA.tile([P, P], FP, tag="pA")
                    for s in range(nsub):
                        nc.tensor.matmul(
                            agg2T[:], lhsT=gts[:, (so + s) * F1:(so + s + 1) * F1],
                            rhs=s2g[:, so + s, :],
                            start=(s == 0), stop=(s == nsub - 1))
                    agg2Ts = pool.tile([P, P], BF, tag="agg2Ts")
                    nc.vector.tensor_copy(out=agg2Ts[:], in_=agg2T[:])
                    h2p = psH.tile([P, F2], FP, tag="pB")
                    nc.tensor.matmul(h2p[:], lhsT=agg2Ts[:], rhs=wrel2[:],
                                     start=True, stop=False)
                    nc.tensor.matmul(h2p[:], lhsT=h1TVall[:, t * P:(t + 1) * P],
                                     rhs=wroot2[:], start=False, stop=True)
                    h2s = pool.tile([P, F2], BF, tag="h2s")
                    if t % 2 == 0:
                        nc.scalar.activation(h2s[:], h2p[:], AF.Relu)
                    else:
                        nc.vector.tensor_relu(out=h2s[:], in_=h2p[:])
                    jj = t % 4
                    if jj == 0:
                        pool_ps = psP.tile([P, F2], FP, tag="pP")
                    nc.tensor.matmul(
                        pool_ps[32 * jj:32 * jj + 32, :], lhsT=s3all[:, t, :],
                        rhs=h2s[:], start=True, stop=True,
                        tile_position=(0, 32 * jj))
                    if jj == 3 or t == NT_V - 1:
                        npart = 32 * (jj + 1)
                        w = t // 4
                        pls = pool.tile([P, F2], FP, tag="pls")
                        nc.vector.tensor_copy(out=pls[:npart, :],
                                              in_=pool_ps[:npart, :])
                        nc.gpsimd.indirect_dma_start(
                            out=emb_d[:, :],
                            out_offset=bass.IndirectOffsetOnAxis(
                                ap=pgall[:npart, w:w + 1], axis=0),
                            in_=pls[:npart, :], in_offset=None,
                            bounds_check=GCOLS, oob_is_err=False)

            tc.strict_bb_all_engine_barrier()

            # ---------------- emb -> embT ----------------
            emball = cpool.tile([P, NT_G, F2], FP, tag="emball")
            nc.sync.dma_start(
                out=emball[:],
                in_=emb_d[:].rearrange("(t p) f -> p t f", p=P))
            embT0 = cpool.tile([P, G_rows], BF, tag="embT0")
            embT1 = cpool.tile([P, G_rows], BF, tag="embT1")
            for t in range(NT_G):
                etb = pool.tile([P, F2], BF, tag="etb")
                nc.vector.tensor_scalar_mul(out=etb[:], in0=emball[:, t, :],
                                            scalar1=invall[:, t:t + 1])
                for half in range(2):
                    tp = psA.tile([P, P], FP, tag="pA")
                    nc.tensor.matmul(tp[:], lhsT=etb[:, half * P:(half + 1) * P],
                                     rhs=ident[:], start=True, stop=True)
                    dst = embT0 if half == 0 else embT1
                    nc.vector.tensor_copy(out=dst[:, t * P:(t + 1) * P], in_=tp[:])

            # ---------------- GRU ----------------
            def batched_gi(xall, wblocks, kt, nm):
                gis = []
                for gate in range(3):
                    gi = cpool.tile([P, GCOLS], FP, tag=f"gi{nm}{gate}")
                    for c0 in range(0, GCOLS, 512):
                        n = min(512, GCOLS - c0)
                        gp = psC.tile([P, 512], FP, tag="pC")
                        for k in range(kt):
                            nc.tensor.matmul(
                                gp[:, :n], lhsT=wblocks[gate * kt + k][:],
                                rhs=xall[k][:, c0:c0 + n],
                                start=(k == 0), stop=(k == kt - 1))
                        nc.vector.tensor_copy(out=gi[:, c0:c0 + n], in_=gp[:, :n])
                    gis.append(gi)
                return gis

            def gru_layer(xall, wih, whh, kt, yout, nm):
                gis = batched_gi(xall, wih, kt, nm)
                h = cpool.tile([P, NSH], BF, tag=f"h_{nm}")
                nc.gpsimd.memset(h[:], 0.0)
                for t in range(TR):
                    ghp = psC.tile([P, 512], FP, tag="pC")
                    for gate in range(3):
                        nc.tensor.matmul(ghp[:, gate * P:(gate + 1) * P],
                                         lhsT=whh[gate][:], rhs=h[:],
                                         start=True, stop=True)

                    def gsl(gate):
                        return gis[gate][:, t::TR][:, :NSH]
                    rs = pool.tile([P, NSH], FP, tag="rs")
                    nc.vector.tensor_add(out=rs[:], in0=gsl(0), in1=ghp[:, 0:P])
                    nc.scalar.activation(rs[:], rs[:], AF.Sigmoid)
                    zs = pool.tile([P, NSH], FP, tag="zs")
                    nc.vector.tensor_add(out=zs[:], in0=gsl(1), in1=ghp[:, P:2 * P])
                    nc.scalar.activation(zs[:], zs[:], AF.Sigmoid)
                    ns_ = pool.tile([P, NSH], FP, tag="ns")
                    nc.vector.tensor_mul(out=ns_[:], in0=rs[:], in1=ghp[:, 2 * P:3 * P])
                    nc.vector.tensor_add(out=ns_[:], in0=ns_[:], in1=gsl(2))
                    nc.scalar.activation(ns_[:], ns_[:], AF.Tanh)
                    hmn = pool.tile([P, NSH], FP, tag="hmn")
                    nc.vector.tensor_sub(out=hmn[:], in0=h[:], in1=ns_[:])
                    nc.vector.tensor_mul(out=hmn[:], in0=hmn[:], in1=zs[:])
                    nc.vector.tensor_add(out=h[:], in0=ns_[:], in1=hmn[:])
                    if yout is not None:
                        nc.vector.tensor_copy(out=yout[:, t::TR][:, :NSH], in_=h[:])
                return h

            y0 = cpool.tile([P, GCOLS], BF, tag="y0")
            gru_layer([embT0, embT1], wih0, whh0, 2, y0, "L0")
            hlast = gru_layer([y0], wih1, whh1, 1, None, "L1")

            lp = psA.tile([P, P], FP, tag="pA")
            nc.tensor.matmul(lp[:12, :NSH], lhsT=dec[:], rhs=hlast[:],
                             start=True, stop=True)
            lo = pool.tile([12, NSH], FP, tag="lo")
            nc.vector.tensor_mul(out=lo[:], in0=lp[:12, :NSH], in1=am[:])
            nc.sync.dma_start(out=out_d[:], in_=lo[:])

    nc.compile()
    return nc


def make_in_map(c, meta, W):
    """Per-core input arrays for run_bass_kernel_spmd."""
    A = c["arrays"]
    bf = lambda a: np.ascontiguousarray(a, dtype=BF16)
    f32 = lambda a: np.ascontiguousarray(a, dtype=np.float32)

    w1s = np.zeros((P, F1), np.float32)
    w1s[0:KSLOT * F_IN] = np.tile(f32(W["c1_wrel"]), (KSLOT, 1))
    w1s[KSLOT * F_IN:SLOT_W] = f32(W["c1_wroot"])
    wih0 = np.stack([np.stack([f32(W["w_ih0"])[g * P:(g + 1) * P, k * P:(k + 1) * P].T
                               for k in range(2)]) for g in range(3)])
    whh0 = np.stack([f32(W["w_hh0"])[g * P:(g + 1) * P, :].T for g in range(3)])
    wih1 = np.stack([f32(W["w_ih1"])[g * P:(g + 1) * P, :].T for g in range(3)])
    whh1 = np.stack([f32(W["w_hh1"])[g * P:(g + 1) * P, :].T for g in range(3)])
    amask = np.broadcast_to(A["amask"][None, :], (12, meta["n_shot_core"]))

    return {
        "xsTV": A["xsTV"],
        "eaTV": A["eaTV"],
        "xs2T": A["xs2T"],
        "ea2T": A["ea2T"],
        "w1s": bf(w1s),
        "s2T": A["s2T"],
        "s3T": A["s3T"],
        "pgT": A["pgT"],
        "invT": A["invT"],
        "amask": f32(amask),
        "ident": bf(np.eye(P, dtype=np.float32)),
        "wrel2": bf(W["c2_wrel"]),
        "wroot2": bf(W["c2_wroot"]),
        "wih0": bf(wih0),
        "whh0": bf(whh0),
        "wih1": bf(wih1),
        "whh1": bf(whh1),
        "dec": bf(W["dec_w"]),
    }


# ------------------------------------------------------------------
_CACHE = {}


def _get_nc(meta):
    key = (meta["NT_V"], meta["E2_slots"], meta["G_rows"],
           tuple(meta["T_sub"]))
    if key not in _CACHE:
        _CACHE[key] = build(meta, num_devices=NC)
    return _CACHE[key]


def kernel(**inputs):
    import sys as _sys
    if "/opt/trn_rl_repo" not in _sys.path:
        _sys.path.insert(0, "/opt/trn_rl_repo")
    from concourse.bass_utils import run_bass_kernel_spmd

    for k in ("c1_b", "c2_b", "b_ih0", "b_hh0", "b_ih1", "b_hh1", "dec_b",
              "empty_emb"):
        assert not np.any(np.asarray(inputs[k])), f"nonzero {k} unsupported"

    cores, meta = prep(inputs)
    W = {k: np.asarray(v, np.float32) for k, v in inputs.items()
         if k not in ("x", "edge_index", "edge_attr", "batch_labels",
                      "label_map", "B")}
    nc = _get_nc(meta)
    in_maps = [make_in_map(c, meta, W) for c in cores]
    res = None
    for attempt in range(6):
        try:
            res = run_bass_kernel_spmd(nc, in_maps, core_ids=list(range(NC)))
            break
        except Exception:
            if attempt == 5:
                raise
    global LAST_RES
    LAST_RES = res
    B = meta["B"]
    out = np.zeros((B, 12), np.float32)
    nsh = meta["n_shot_core"]
    for d in range(NC):
        lg = res.results[d]["out"]          # [12, nsh]
        s = d + NC * np.arange(nsh)
        out[s[s < B]] = lg.T[s < B]
    return out


# revision 25
# speedup vs baseline: 1.0160x; 1.0160x over previous
"""Host-side sharding/prep + Bass device program for nn_BBGRUDecoder.

Host does index manipulation / data layout only; the device kernel does all
model arithmetic.

v4 design:
- conv1 slot arrays carry the root feature as slot KSLOT (weight 1.0) and are
  shipped pre-transposed [SLOT_W, rows] so the conv1 matmul needs no
  on-device transpose and no tree-reduce.
- conv2 does NOT gather h1 rows (SWDGE gather costs ~10ns/row on gpsimd).
  Instead the host lays out each edge's SOURCE-node slot data edge-major
  ([SLOT_W, E2_slots]) and the device recomputes h1 per edge subtile with one
  extra matmul+relu. No gather, no compaction, no DRAM h1 tables.
- conv1 computes h1 only for local V tiles (root term), kept resident in SBUF.
- s2 (edge->dst scatter weights) shipped transposed [128, E2_slots] for wide
  contiguous loads; s3/pool_gid/inv_cnt/emb are single-DMA preloads.
"""
import numpy as np
import ml_dtypes

BF16 = np.dtype(ml_dtypes.bfloat16)
NC = 8
P = 128
KSLOT = 16       # conv1 in-edge slots per node (max in-degree 13)
KSLOT2 = 17      # + root slot
F_IN = 5
SLOT_W = KSLOT2 * F_IN   # 85
F1 = 128
F2 = 256
HID = 128
TR = 10          # rounds per shot
MAXG_TILE = 32   # max graphs per node-tile (pool S3 width)
OCT = 16         # conv1 tiles per input DMA


def _pack_groups(sizes, cap_items, cap_groups):
    """Greedy-pack consecutive groups (each <=cap_items items) into tiles of
    <=cap_items items and <=cap_groups groups."""
    tiles = []
    i = 0
    n = len(sizes)
    while i < n:
        items = 0
        g = 0
        while i + g < n and g < cap_groups and items + sizes[i + g] <= cap_items:
            items += sizes[i + g]
            g += 1
        assert g > 0, f"group {i} size {sizes[i]} exceeds cap {cap_items}"
        tiles.append((i, g, items))
        i += g
    return tiles


def prep(inputs):
    x = np.asarray(inputs["x"], np.float32)
    ei = np.asarray(inputs["edge_index"], np.int64)
    ea = np.asarray(inputs["edge_attr"], np.float32)
    bl = np.asarray(inputs["batch_labels"], np.int64)
    lm = np.asarray(inputs["label_map"], np.int64)
    B = int(inputs["B"])
    NN = x.shape[0]
    src_g, dst_g = ei[0], ei[1]
    shot_of, round_of = lm[:, 0], lm[:, 1]
    n_shot_core = (B + NC - 1) // NC          # 128 shots per core
    GCOLS = n_shot_core * TR                  # 1280 graph-columns per core
    deg = np.bincount(dst_g, minlength=NN)
    assert deg.max() <= KSLOT

    # ---- global conv1 slot data [NN, KSLOT2, F_IN]; slot KSLOT = root ----
    xs_all = np.zeros((NN, KSLOT2, F_IN), np.float32)
    ea_all = np.zeros((NN, KSLOT2, F_IN), np.float32)
    xs_all[:, KSLOT] = x
    ea_all[:, KSLOT] = 1.0
    order = np.argsort(dst_g, kind="stable")
    ds = dst_g[order]
    sl = np.arange(len(ds)) - np.searchsorted(ds, ds)   # slot within dst run
    xs_all[ds, sl] = x[src_g[order]]
    ea_all[ds, sl] = ea[order][:, None]
    xs_flat = xs_all.reshape(NN, SLOT_W).astype(BF16)
    ea_flat = ea_all.reshape(NN, SLOT_W).astype(BF16)

    node_g = bl
    node_core = (shot_of[node_g] % NC).astype(np.int64)

    cores = []
    for d in range(NC):
        V = np.nonzero(node_core == d)[0]          # ascending node ids
        gids, gstart, gcnt = np.unique(node_g[V], return_index=True, return_counts=True)
        s_idx = shot_of[gids] // NC
        gcol = s_idx * TR + round_of[gids]
        tiles = _pack_groups(gcnt.tolist(), P, MAXG_TILE)
        NT_V = len(tiles)
        vpos = np.full(NN, -1, np.int64)
        packed_rows = []
        tile_graphs = []
        for (g0, ng, ni) in tiles:
            rows = []
            for k in range(g0, g0 + ng):
                rows.append(V[gstart[k]:gstart[k] + gcnt[k]])
            rows = np.concatenate(rows)
            packed_rows.append(rows)
            tile_graphs.append((gcol[g0:g0 + ng], gcnt[g0:g0 + ng]))
        for t, rows in enumerate(packed_rows):
            vpos[rows] = t * P + np.arange(len(rows))

        E = np.nonzero(node_core[dst_g] == d)[0]
        cores.append(dict(
            d=d, V=V, NT_V=NT_V, packed_rows=packed_rows,
            tile_graphs=tile_graphs, vpos=vpos,
            e_src=src_g[E], e_dst=dst_g[E], e_ea=ea[E],
            gids=gids, gcol=gcol, gcnt=gcnt,
        ))

    # ---- shared static shapes ----
    NT_V = max(c["NT_V"] for c in cores)
    V_pad = NT_V * P

    T_sub = np.zeros(NT_V, np.int64)
    for c in cores:
        for t in range(NT_V):
            if t < c["NT_V"]:
                ne = int(deg[c["packed_rows"][t]].sum())
            else:
                ne = 0
            T_sub[t] = max(T_sub[t], -(-ne // P) if ne else 1)
    E2_slots = int(T_sub.sum()) * P
    NW = -(-NT_V // 4)     # scatter windows (4 tiles each)

    meta = dict(NT_V=NT_V, V_pad=V_pad, T_sub=T_sub.tolist(),
                E2_slots=E2_slots, GCOLS=GCOLS, G_rows=-(-(GCOLS + 1) // P) * P,
                n_shot_core=n_shot_core, B=B, NW=NW)

    # ---- per-core padded arrays ----
    for c in cores:
        vpos = c["vpos"]
        # conv1 V slot data, transposed [SLOT_W, V_pad]
        xsV = np.zeros((V_pad, SLOT_W), BF16)
        eaV = np.zeros((V_pad, SLOT_W), BF16)
        for t, rows in enumerate(c["packed_rows"]):
            xsV[t * P:t * P + len(rows)] = xs_flat[rows]
            eaV[t * P:t * P + len(rows)] = ea_flat[rows]
        xsTV = np.ascontiguousarray(xsV.T)
        eaTV = np.ascontiguousarray(eaV.T)

        # conv2: edge-major src slot data + s2 scatter weights, per tile
        xs2 = np.zeros((E2_slots, SLOT_W), BF16)
        ea2 = np.zeros((E2_slots, SLOT_W), BF16)
        s2 = np.zeros((E2_slots // P, P, P), np.float32)
        st = 0
        for t in range(NT_V):
            nsub = int(T_sub[t])
            if t < c["NT_V"]:
                sel = np.nonzero((vpos[c["e_dst"]] >= t * P) &
                                 (vpos[c["e_dst"]] < t * P + P))[0]
                es, ed, ew = c["e_src"][sel], c["e_dst"][sel], c["e_ea"][sel]
                ne = len(es)
                xs2[st * P:st * P + ne] = xs_flat[es]
                ea2[st * P:st * P + ne] = ea_flat[es]
                loc = vpos[ed] - t * P
                s2[st + np.arange(ne) // P, np.arange(ne) % P, loc] = ew
            st += nsub
        assert st * P == E2_slots
        xs2T = np.ascontiguousarray(xs2.T)
        ea2T = np.ascontiguousarray(ea2.T)
        s2T = np.ascontiguousarray(
            s2.transpose(1, 0, 2).reshape(P, E2_slots).astype(BF16))

        # pool S3 / graph ids / inv counts
        s3 = np.zeros((NT_V, P, MAXG_TILE), np.float32)
        pool_gid = np.full((NT_V, MAXG_TILE), meta["GCOLS"] + 100, np.int64)
        for t in range(c["NT_V"]):
            gcols, gcnts = c["tile_graphs"][t]
            off = 0
            for j, (gc, n) in enumerate(zip(gcols, gcnts)):
                s3[t, off:off + n, j] = 1
                pool_gid[t, j] = gc
                off += n
        s3T = np.ascontiguousarray(s3.transpose(1, 0, 2).astype(BF16))
        pg_pad = np.full((NW * 4, MAXG_TILE), meta["GCOLS"] + 100, np.int64)
        pg_pad[:NT_V] = pool_gid
        pgT = np.ascontiguousarray(pg_pad.reshape(NW, P).T.astype(np.int32))
        inv_cnt = np.zeros(meta["G_rows"], np.float32)
        inv_cnt[c["gcol"]] = 1.0 / np.maximum(c["gcnt"], 1)
        invT = np.ascontiguousarray(inv_cnt.reshape(-1, P).T)

        amask = np.zeros(n_shot_core, np.float32)
        amask[(shot_of[c["gids"]] // NC)] = 1.0

        c["arrays"] = dict(
            xsTV=xsTV, eaTV=eaTV, xs2T=xs2T, ea2T=ea2T, s2T=s2T,
            s3T=s3T, pgT=pgT, invT=invT, amask=amask,
        )
    return cores, meta


# ======================================================
"""Bass/Tile device program (per-core SPMD)."""
import concourse.bass as bass
import concourse.bacc as bacc
import concourse.mybir as mybir
from concourse.tile import TileContext


BF = mybir.dt.bfloat16
FP = mybir.dt.float32
AF = mybir.ActivationFunctionType


def build(meta, num_devices=8):
    NT_V = meta["NT_V"]
    V_pad = meta["V_pad"]
    T_sub = meta["T_sub"]
    E2_slots = meta["E2_slots"]
    GCOLS, G_rows = meta["GCOLS"], meta["G_rows"]
    NSH = meta["n_shot_core"]
    NW = meta["NW"]
    NT_G = G_rows // P

    nc = bacc.Bacc("TRN2", target_bir_lowering=False, debug=False,
                   num_devices=num_devices)

    def inp(name, shape, dt):
        return nc.dram_tensor(name, shape, dt, kind="ExternalInput")

    xsTV_d = inp("xsTV", [SLOT_W, V_pad], BF)
    eaTV_d = inp("eaTV", [SLOT_W, V_pad], BF)
    xs2T_d = inp("xs2T", [SLOT_W, E2_slots], BF)
    ea2T_d = inp("ea2T", [SLOT_W, E2_slots], BF)
    w1s_d = inp("w1s", [P, F1], BF)
    s2T_d = inp("s2T", [P, E2_slots], BF)
    s3T_d = inp("s3T", [P, NT_V, MAXG_TILE], BF)
    pgT_d = inp("pgT", [P, NW], mybir.dt.int32)
    invT_d = inp("invT", [P, NT_G], FP)
    amask_d = inp("amask", [12, NSH], FP)
    ident_d = inp("ident", [P, P], BF)
    wrel2_d = inp("wrel2", [P, F2], BF)
    wroot2_d = inp("wroot2", [P, F2], BF)
    wih0_d = inp("wih0", [3, 2, P, P], BF)
    whh0_d = inp("whh0", [3, P, P], BF)
    wih1_d = inp("wih1", [3, P, P], BF)
    whh1_d = inp("whh1", [3, P, P], BF)
    dec_d = inp("dec", [P, 12], BF)
    out_d = nc.dram_tensor("out", [12, NSH], FP, kind="ExternalOutput")

    emb_d = nc.dram_tensor("emb", [G_rows, F2], FP, kind="Internal")

    with TileContext(nc) as tc:
        with (
            tc.tile_pool(name="const", bufs=1) as cpool,
            tc.tile_pool(name="sb", bufs=3) as pool,
            tc.tile_pool(name="big", bufs=3) as bigp,
            tc.tile_pool(name="psA", bufs=3, space="PSUM") as psA,
            tc.tile_pool(name="psH", bufs=2, space="PSUM") as psH,
            tc.tile_pool(name="psP", bufs=1, space="PSUM") as psP,
            tc.tile_pool(name="psC", bufs=2, space="PSUM") as psC,
        ):
            # ---------------- constants / preloads ----------------
            ident = cpool.tile([P, P], BF, tag="ident")
            nc.sync.dma_start(out=ident[:], in_=ident_d[:])
            w1s = cpool.tile([P, F1], BF, tag="w1s")
            nc.sync.dma_start(out=w1s[:], in_=w1s_d[:])
            wrel2 = cpool.tile([P, F2], BF, tag="wrel2")
            nc.sync.dma_start(out=wrel2[:], in_=wrel2_d[:])
            wroot2 = cpool.tile([P, F2], BF, tag="wroot2")
            nc.sync.dma_start(out=wroot2[:], in_=wroot2_d[:])

            wih0 = []
            for gate in range(3):
                for k in range(2):
                    wt = cpool.tile([P, P], BF, tag=f"wih0_{gate}_{k}")
                    nc.sync.dma_start(out=wt[:], in_=wih0_d[gate, k])
                    wih0.append(wt)

            def load3(dram, nm):
                ts = []
                for i in range(3):
                    wt = cpool.tile([P, P], BF, tag=f"{nm}{i}")
                    nc.sync.dma_start(out=wt[:], in_=dram[i])
                    ts.append(wt)
                return ts

            whh0 = load3(whh0_d, "whh0")
            wih1 = load3(wih1_d, "wih1")
            whh1 = load3(whh1_d, "whh1")
            dec = cpool.tile([P, 12], BF, tag="dec")
            nc.sync.dma_start(out=dec[:], in_=dec_d[:])
            am = cpool.tile([12, NSH], FP, tag="am")
            nc.sync.dma_start(out=am[:], in_=amask_d[:])
            s3all = cpool.tile([P, NT_V, MAXG_TILE], BF, tag="s3all")
            nc.sync.dma_start(out=s3all[:], in_=s3T_d[:])
            pgall = cpool.tile([P, NW], mybir.dt.int32, tag="pgall")
            nc.sync.dma_start(out=pgall[:], in_=pgT_d[:])
            invall = cpool.tile([P, NT_G], FP, tag="invall")
            nc.sync.dma_start(out=invall[:], in_=invT_d[:])

            # zero emb via gpsimd queue so the later indirect scatters
            # (same SWDGE FIFO) are ordered after it without a barrier
            zt = cpool.tile([P, F2], FP, tag="zero")
            nc.gpsimd.memset(zt[:], 0.0)
            for t in range(NT_G):
                nc.gpsimd.dma_start(out=emb_d[t * P:(t + 1) * P, :], in_=zt[:])

            # h1^T of V tiles stays resident for conv2's root term
            # (features on partitions, node columns)
            h1TVall = cpool.tile([P, V_pad], BF, tag="h1TVall")

            # ---------------- conv1 over V tiles ----------------
            n_oct = (NT_V + OCT - 1) // OCT
            for o in range(n_oct):
                t0 = o * OCT
                nt = min(OCT, NT_V - t0)
                cols = slice(t0 * P, (t0 + nt) * P)
                xs_t = pool.tile([SLOT_W, OCT * P], BF, tag="xs1")
                nc.sync.dma_start(out=xs_t[:, :nt * P], in_=xsTV_d[:, cols])
                ea_t = pool.tile([SLOT_W, OCT * P], BF, tag="ea1")
                nc.sync.dma_start(out=ea_t[:, :nt * P], in_=eaTV_d[:, cols])
                msgT = pool.tile([SLOT_W, OCT * P], BF, tag="msg1")
                nc.vector.tensor_mul(out=msgT[:, :nt * P], in0=xs_t[:, :nt * P],
                                     in1=ea_t[:, :nt * P])
                for g0 in range(0, nt, 4):
                    tg = t0 + g0
                    ng = min(4, nt - g0)
                    h1p = psC.tile([P, 4 * P], FP, tag="pC")
                    nc.tensor.matmul(
                        h1p[:, :ng * P], lhsT=w1s[:SLOT_W, :],
                        rhs=msgT[:, g0 * P:(g0 + ng) * P],
                        start=True, stop=True)
                    dst = h1TVall[:, tg * P:(tg + ng) * P]
                    if (o + g0 // 4) % 2 == 0:
                        nc.scalar.activation(dst, h1p[:, :ng * P], AF.Relu)
                    else:
                        nc.vector.tensor_relu(out=dst, in_=h1p[:, :ng * P])

            # ---------------- conv2 + pool (gather-free) ----------------
            GG = 4
            sub_start = np.concatenate([[0], np.cumsum(T_sub)]).astype(int)
            plan = []
            t = 0
            while t < NT_V:
                te = t
                while te < NT_V and sub_start[te + 1] - sub_start[t] <= GG:
                    te += 1
                plan.append((t, te))
                t = te
            pool_ps = None
            for (ta, te) in plan:
                so0 = int(sub_start[ta])
                ns = int(sub_start[te]) - so0
                ecols = slice(so0 * P, (so0 + ns) * P)
                xs2_t = pool.tile([SLOT_W, GG * P], BF, tag="xs2")
                nc.sync.dma_start(out=xs2_t[:, :ns * P], in_=xs2T_d[:, ecols])
                ea2_t = pool.tile([SLOT_W, GG * P], BF, tag="ea2")
                nc.sync.dma_start(out=ea2_t[:, :ns * P], in_=ea2T_d[:, ecols])
                msg2 = pool.tile([SLOT_W, GG * P], BF, tag="msg2")
                nc.vector.tensor_mul(out=msg2[:, :ns * P], in0=xs2_t[:, :ns * P],
                                     in1=ea2_t[:, :ns * P])
                s2g = bigp.tile([P, GG, P], BF, tag="s2g")
                nc.sync.dma_start(
                    out=s2g[:, :ns, :],
                    in_=s2T_d[:, ecols].rearrange("p (s q) -> p s q", q=P))
                # h1 of edge sources, 4 subtiles per PSUM bank
                gts = bigp.tile([P, GG * F1], BF, tag="gts")
                for sb in range(0, ns, 4):
                    nb = min(4, ns - sb)
                    hep = psC.tile([P, 4 * P], FP, tag="pC")
                    for k in range(nb):
                        nc.tensor.matmul(
                            hep[:, k * P:(k + 1) * P],
                            lhsT=msg2[:, (sb + k) * P:(sb + k + 1) * P],
                            rhs=w1s[:SLOT_W, :],
                            start=True, stop=True)
                    dst = gts[:, sb * F1:(sb + nb) * F1]
                    if (sb // 4) % 2 == 0:
                        nc.scalar.activation(dst, hep[:, :nb * P], AF.Relu)
                    else:
                        nc.vector.tensor_relu(out=dst, in_=hep[:, :nb * P])
                for t in range(ta, te):
                    so = int(sub_start[t]) - so0
                    nsub = T_sub[t]
                    agg2T = psA.tile([P, P], FP, tag="pA")
                    for s in range(nsub):
                        nc.tensor.matmul(
                            agg2T[:], lhsT=gts[:, (so + s) * F1:(so + s + 1) * F1],
                            rhs=s2g[:, so + s, :],
                            start=(s == 0), stop=(s == nsub - 1))
                    agg2Ts = pool.tile([P, P], BF, tag="agg2Ts")
                    nc.vector.tensor_copy(out=agg2Ts[:], in_=agg2T[:])
                    h2p = psH.tile([P, F2], FP, tag="pB")
                    nc.tensor.matmul(h2p[:], lhsT=agg2Ts[:], rhs=wrel2[:],
                                     start=True, stop=False)
                    nc.tensor.matmul(h2p[:], lhsT=h1TVall[:, t * P:(t + 1) * P],
                                     rhs=wroot2[:], start=False, stop=True)
                    h2s = pool.tile([P, F2], BF, tag="h2s")
                    if t % 2 == 0:
                        nc.scalar.activation(h2s[:], h2p[:], AF.Relu)
                    else:
                        nc.vector.tensor_relu(out=h2s[:], in_=h2p[:])
                    jj = t % 4
                    if jj == 0:
                        pool_ps = psP.tile([P, F2], FP, tag="pP")
                    nc.tensor.matmul(
                        pool_ps[32 * jj:32 * jj + 32, :], lhsT=s3all[:, t, :],
                        rhs=h2s[:], start=True, stop=True,
                        tile_position=(0, 32 * jj))
                    if jj == 3 or t == NT_V - 1:
                        npart = 32 * (jj + 1)
                        w = t // 4
                        pls = pool.tile([P, F2], FP, tag="pls")
                        nc.vector.tensor_copy(out=pls[:npart, :],
                                              in_=pool_ps[:npart, :])
                        nc.gpsimd.indirect_dma_start(
                            out=emb_d[:, :],
                            out_offset=bass.IndirectOffsetOnAxis(
                                ap=pgall[:npart, w:w + 1], axis=0),
                            in_=pls[:npart, :], in_offset=None,
                            bounds_check=GCOLS, oob_is_err=False)

            tc.strict_bb_all_engine_barrier()

            # ---------------- emb -> embT ----------------
            emball = cpool.tile([P, NT_G, F2], FP, tag="emball")
            nc.sync.dma_start(
                out=emball[:],
                in_=emb_d[:].rearrange("(t p) f -> p t f", p=P))
            embT0 = cpool.tile([P, G_rows], BF, tag="embT0")
            embT1 = cpool.tile([P, G_rows], BF, tag="embT1")
            for t in range(NT_G):
                etb = pool.tile([P, F2], BF, tag="etb")
                nc.vector.tensor_scalar_mul(out=etb[:], in0=emball[:, t, :],
                                            scalar1=invall[:, t:t + 1])
                for half in range(2):
                    tp = psA.tile([P, P], FP, tag="pA")
                    nc.tensor.matmul(tp[:], lhsT=etb[:, half * P:(half + 1) * P],
                                     rhs=ident[:], start=True, stop=True)
                    dst = embT0 if half == 0 else embT1
                    nc.vector.tensor_copy(out=dst[:, t * P:(t + 1) * P], in_=tp[:])

            # ---------------- GRU ----------------
            def batched_gi(xall, wblocks, kt, nm):
                gis = []
                for gate in range(3):
                    gi = cpool.tile([P, GCOLS], FP, tag=f"gi{nm}{gate}")
                    for c0 in range(0, GCOLS, 512):
                        n = min(512, GCOLS - c0)
                        gp = psC.tile([P, 512], FP, tag="pC")
                        for k in range(kt):
                            nc.tensor.matmul(
                                gp[:, :n], lhsT=wblocks[gate * kt + k][:],
                                rhs=xall[k][:, c0:c0 + n],
                                start=(k == 0), stop=(k == kt - 1))
                        nc.vector.tensor_copy(out=gi[:, c0:c0 + n], in_=gp[:, :n])
                    gis.append(gi)
                return gis

            def gru_layer(xall, wih, whh, kt, yout, nm):
                gis = batched_gi(xall, wih, kt, nm)
                h = cpool.tile([P, NSH], BF, tag=f"h_{nm}")
                nc.gpsimd.memset(h[:], 0.0)
                for t in range(TR):
                    ghp = psC.tile([P, 512], FP, tag="pC")
                    for gate in range(3):
                        nc.tensor.matmul(ghp[:, gate * P:(gate + 1) * P],
                                         lhsT=whh[gate][:], rhs=h[:],
                                         start=True, stop=True)

                    def gsl(gate):
                        return gis[gate][:, t::TR][:, :NSH]
                    rs = pool.tile([P, NSH], FP, tag="rs")
                    nc.vector.tensor_add(out=rs[:], in0=gsl(0), in1=ghp[:, 0:P])
                    nc.scalar.activation(rs[:], rs[:], AF.Sigmoid)
                    zs = pool.tile([P, NSH], FP, tag="zs")
                    nc.vector.tensor_add(out=zs[:], in0=gsl(1), in1=ghp[:, P:2 * P])
                    nc.scalar.activation(zs[:], zs[:], AF.Sigmoid)
                    ns_ = pool.tile([P, NSH], FP, tag="ns")
                    nc.vector.tensor_mul(out=ns_[:], in0=rs[:], in1=ghp[:, 2 * P:3 * P])
                    nc.vector.tensor_add(out=ns_[:], in0=ns_[:], in1=gsl(2))
                    nc.scalar.activation(ns_[:], ns_[:], AF.Tanh)
                    hmn = pool.tile([P, NSH], FP, tag="hmn")
                    nc.vector.tensor_sub(out=hmn[:], in0=h[:], in1=ns_[:])
                    nc.vector.tensor_mul(out=hmn[:], in0=hmn[:], in1=zs[:])
                    nc.vector.tensor_add(out=h[:], in0=ns_[:], in1=hmn[:])
                    if yout is not None:
                        nc.vector.tensor_copy(out=yout[:, t::TR][:, :NSH], in_=h[:])
                return h

            y0 = cpool.tile([P, GCOLS], BF, tag="y0")
            gru_layer([embT0, embT1], wih0, whh0, 2, y0, "L0")
            hlast = gru_layer([y0], wih1, whh1, 1, None, "L1")

            lp = psA.tile([P, P], FP, tag="pA")
            nc.tensor.matmul(lp[:12, :NSH], lhsT=dec[:], rhs=hlast[:],
                             start=True, stop=True)
            lo = pool.tile([12, NSH], FP, tag="lo")
            nc.vector.tensor_mul(out=lo[:], in0=lp[:12, :NSH], in1=am[:])
            nc.sync.dma_start(out=out_d[:], in_=lo[:])

    nc.compile()
    return nc


def make_in_map(c, meta, W):
    """Per-core input arrays for run_bass_kernel_spmd."""
    A = c["arrays"]
    bf = lambda a: np.ascontiguousarray(a, dtype=BF16)
    f32 = lambda a: np.ascontiguousarray(a, dtype=np.float32)

    w1s = np.zeros((P, F1), np.float32)
    w1s[0:KSLOT * F_IN] = np.tile(f32(W["c1_wrel"]), (KSLOT, 1))
    w1s[KSLOT * F_IN:SLOT_W] = f32(W["c1_wroot"])
    wih0 = np.stack([np.stack([f32(W["w_ih0"])[g * P:(g + 1) * P, k * P:(k + 1) * P].T
                               for k in range(2)]) for g in range(3)])
    whh0 = np.stack([f32(W["w_hh0"])[g * P:(g + 1) * P, :].T for g in range(3)])
    wih1 = np.stack([f32(W["w_ih1"])[g * P:(g + 1) * P, :].T for g in range(3)])
    whh1 = np.stack([f32(W["w_hh1"])[g * P:(g + 1) * P, :].T for g in range(3)])
    amask = np.broadcast_to(A["amask"][None, :], (12, meta["n_shot_core"]))

    return {
        "xsTV": A["xsTV"],
        "eaTV": A["eaTV"],
        "xs2T": A["xs2T"],
        "ea2T": A["ea2T"],
        "w1s": bf(w1s),
        "s2T": A["s2T"],
        "s3T": A["s3T"],
        "pgT": A["pgT"],
        "invT": A["invT"],
        "amask": f32(amask),
        "ident": bf(np.eye(P, dtype=np.float32)),
        "wrel2": bf(W["c2_wrel"]),
        "wroot2": bf(W["c2_wroot"]),
        "wih0": bf(wih0),
        "whh0": bf(whh0),
        "wih1": bf(wih1),
        "whh1": bf(whh1),
        "dec": bf(W["dec_w"]),
    }


# ------------------------------------------------------------------
_CACHE = {}


def _get_nc(meta):
    key = (meta["NT_V"], meta["E2_slots"], meta["G_rows"],
           tuple(meta["T_sub"]))
    if key not in _CACHE:
        _CACHE[key] = build(meta, num_devices=NC)
    return _CACHE[key]


def kernel(**inputs):
    import sys as _sys
    if "/opt/trn_rl_repo" not in _sys.path:
        _sys.path.insert(0, "/opt/trn_rl_repo")
    from concourse.bass_utils import run_bass_kernel_spmd

    for k in ("c1_b", "c2_b", "b_ih0", "b_hh0", "b_ih1", "b_hh1", "dec_b",
              "empty_emb"):
        assert not np.any(np.asarray(inputs[k])), f"nonzero {k} unsupported"

    cores, meta = prep(inputs)
    W = {k: np.asarray(v, np.float32) for k, v in inputs.items()
         if k not in ("x", "edge_index", "edge_attr", "batch_labels",
                      "label_map", "B")}
    nc = _get_nc(meta)
    in_maps = [make_in_map(c, meta, W) for c in cores]
    res = None
    for attempt in range(6):
        try:
            res = run_bass_kernel_spmd(nc, in_maps, core_ids=list(range(NC)))
            break
        except Exception:
            if attempt == 5:
                raise
    global LAST_RES
    LAST_RES = res
    B = meta["B"]
    out = np.zeros((B, 12), np.float32)
    nsh = meta["n_shot_core"]
    for d in range(NC):
        lg = res.results[d]["out"]          # [12, nsh]
        s = d + NC * np.arange(nsh)
        out[s[s < B]] = lg.T[s < B]
    return out


# revision 26
# speedup vs baseline: 1.1259x; 1.1082x over previous
"""Host-side sharding/prep + Bass device program for nn_BBGRUDecoder.

Host does index manipulation / data layout only; the device kernel does all
model arithmetic.

v4 design:
- conv1 slot arrays carry the root feature as slot KSLOT (weight 1.0) and are
  shipped pre-transposed [SLOT_W, rows] so the conv1 matmul needs no
  on-device transpose and no tree-reduce.
- conv2 does NOT gather h1 rows (SWDGE gather costs ~10ns/row on gpsimd).
  Instead the host lays out each edge's SOURCE-node slot data edge-major
  ([SLOT_W, E2_slots]) and the device recomputes h1 per edge subtile with one
  extra matmul+relu. No gather, no compaction, no DRAM h1 tables.
- conv1 computes h1 only for local V tiles (root term), kept resident in SBUF.
- s2 (edge->dst scatter weights) shipped transposed [128, E2_slots] for wide
  contiguous loads; s3/pool_gid/inv_cnt/emb are single-DMA preloads.
"""
import numpy as np
import ml_dtypes

BF16 = np.dtype(ml_dtypes.bfloat16)
NC = 8
P = 128
KSLOT = 16       # conv1 in-edge slots per node (max in-degree 13)
KSLOT2 = 17      # + root slot
F_IN = 5
SLOT_W = KSLOT2 * F_IN   # 85
F1 = 128
F2 = 256
HID = 128
TR = 10          # rounds per shot
MAXG_TILE = 32   # max graphs per node-tile (pool S3 width)
OCT = 16         # conv1 tiles per input DMA


def _pack_groups(sizes, cap_items, cap_groups):
    """Greedy-pack consecutive groups (each <=cap_items items) into tiles of
    <=cap_items items and <=cap_groups groups."""
    tiles = []
    i = 0
    n = len(sizes)
    while i < n:
        items = 0
        g = 0
        while i + g < n and g < cap_groups and items + sizes[i + g] <= cap_items:
            items += sizes[i + g]
            g += 1
        assert g > 0, f"group {i} size {sizes[i]} exceeds cap {cap_items}"
        tiles.append((i, g, items))
        i += g
    return tiles


def prep(inputs):
    x = np.asarray(inputs["x"], np.float32)
    ei = np.asarray(inputs["edge_index"], np.int64)
    ea = np.asarray(inputs["edge_attr"], np.float32)
    bl = np.asarray(inputs["batch_labels"], np.int64)
    lm = np.asarray(inputs["label_map"], np.int64)
    B = int(inputs["B"])
    NN = x.shape[0]
    src_g, dst_g = ei[0], ei[1]
    shot_of, round_of = lm[:, 0], lm[:, 1]
    n_shot_core = (B + NC - 1) // NC          # 128 shots per core
    GCOLS = n_shot_core * TR                  # 1280 graph-columns per core
    deg = np.bincount(dst_g, minlength=NN)
    assert deg.max() <= KSLOT

    # ---- global conv1 slot data [NN, KSLOT2, F_IN]; slot KSLOT = root ----
    xs_all = np.zeros((NN, KSLOT2, F_IN), np.float32)
    ea_all = np.zeros((NN, KSLOT2, F_IN), np.float32)
    xs_all[:, KSLOT] = x
    ea_all[:, KSLOT] = 1.0
    order = np.argsort(dst_g, kind="stable")
    ds = dst_g[order]
    sl = np.arange(len(ds)) - np.searchsorted(ds, ds)   # slot within dst run
    xs_all[ds, sl] = x[src_g[order]]
    ea_all[ds, sl] = ea[order][:, None]
    xs_flat = xs_all.reshape(NN, SLOT_W).astype(BF16)
    ea_flat = ea_all.reshape(NN, SLOT_W).astype(BF16)

    node_g = bl
    node_core = (shot_of[node_g] % NC).astype(np.int64)

    cores = []
    for d in range(NC):
        V = np.nonzero(node_core == d)[0]          # ascending node ids
        gids, gstart, gcnt = np.unique(node_g[V], return_index=True, return_counts=True)
        s_idx = shot_of[gids] // NC
        gcol = s_idx * TR + round_of[gids]
        tiles = _pack_groups(gcnt.tolist(), P, MAXG_TILE)
        NT_V = len(tiles)
        vpos = np.full(NN, -1, np.int64)
        packed_rows = []
        tile_graphs = []
        for (g0, ng, ni) in tiles:
            rows = []
            for k in range(g0, g0 + ng):
                rows.append(V[gstart[k]:gstart[k] + gcnt[k]])
            rows = np.concatenate(rows)
            packed_rows.append(rows)
            tile_graphs.append((gcol[g0:g0 + ng], gcnt[g0:g0 + ng]))
        for t, rows in enumerate(packed_rows):
            vpos[rows] = t * P + np.arange(len(rows))

        E = np.nonzero(node_core[dst_g] == d)[0]
        cores.append(dict(
            d=d, V=V, NT_V=NT_V, packed_rows=packed_rows,
            tile_graphs=tile_graphs, vpos=vpos,
            e_src=src_g[E], e_dst=dst_g[E], e_ea=ea[E],
            gids=gids, gcol=gcol, gcnt=gcnt,
        ))

    # ---- shared static shapes ----
    NT_V = max(c["NT_V"] for c in cores)
    V_pad = NT_V * P

    T_sub = np.zeros(NT_V, np.int64)
    for c in cores:
        for t in range(NT_V):
            if t < c["NT_V"]:
                ne = int(deg[c["packed_rows"][t]].sum())
            else:
                ne = 0
            T_sub[t] = max(T_sub[t], -(-ne // P) if ne else 1)
    E2_slots = int(T_sub.sum()) * P
    NW = -(-NT_V // 4)     # scatter windows (4 tiles each)

    meta = dict(NT_V=NT_V, V_pad=V_pad, T_sub=T_sub.tolist(),
                E2_slots=E2_slots, GCOLS=GCOLS, G_rows=-(-(GCOLS + 1) // P) * P,
                n_shot_core=n_shot_core, B=B, NW=NW)

    # ---- per-core padded arrays ----
    for c in cores:
        vpos = c["vpos"]
        # conv1 V slot data, transposed [SLOT_W, V_pad]
        xsV = np.zeros((V_pad, SLOT_W), BF16)
        eaV = np.zeros((V_pad, SLOT_W), BF16)
        for t, rows in enumerate(c["packed_rows"]):
            xsV[t * P:t * P + len(rows)] = xs_flat[rows]
            eaV[t * P:t * P + len(rows)] = ea_flat[rows]
        xsTV = np.ascontiguousarray(xsV.T)
        eaTV = np.ascontiguousarray(eaV.T)

        # conv2: edge-major src slot data + s2 scatter weights, per tile
        xs2 = np.zeros((E2_slots, SLOT_W), BF16)
        ea2 = np.zeros((E2_slots, SLOT_W), BF16)
        s2 = np.zeros((E2_slots // P, P, P), np.float32)
        st = 0
        for t in range(NT_V):
            nsub = int(T_sub[t])
            if t < c["NT_V"]:
                sel = np.nonzero((vpos[c["e_dst"]] >= t * P) &
                                 (vpos[c["e_dst"]] < t * P + P))[0]
                es, ed, ew = c["e_src"][sel], c["e_dst"][sel], c["e_ea"][sel]
                ne = len(es)
                xs2[st * P:st * P + ne] = xs_flat[es]
                ea2[st * P:st * P + ne] = ea_flat[es]
                loc = vpos[ed] - t * P
                s2[st + np.arange(ne) // P, np.arange(ne) % P, loc] = ew
            st += nsub
        assert st * P == E2_slots
        xs2T = np.ascontiguousarray(xs2.T)
        ea2T = np.ascontiguousarray(ea2.T)
        s2T = np.ascontiguousarray(
            s2.transpose(1, 0, 2).reshape(P, E2_slots).astype(BF16))

        # pool S3 / graph ids / inv counts
        s3 = np.zeros((NT_V, P, MAXG_TILE), np.float32)
        pool_gid = np.full((NT_V, MAXG_TILE), meta["GCOLS"] + 100, np.int64)
        for t in range(c["NT_V"]):
            gcols, gcnts = c["tile_graphs"][t]
            off = 0
            for j, (gc, n) in enumerate(zip(gcols, gcnts)):
                s3[t, off:off + n, j] = 1
                pool_gid[t, j] = gc
                off += n
        s3T = np.ascontiguousarray(s3.transpose(1, 0, 2).astype(BF16))
        pg_pad = np.full((NW * 4, MAXG_TILE), meta["GCOLS"] + 100, np.int64)
        pg_pad[:NT_V] = pool_gid
        pgT = np.ascontiguousarray(pg_pad.reshape(NW, P).T.astype(np.int32))
        inv_cnt = np.zeros(meta["G_rows"], np.float32)
        inv_cnt[c["gcol"]] = 1.0 / np.maximum(c["gcnt"], 1)
        invT = np.ascontiguousarray(inv_cnt.reshape(-1, P).T)

        amask = np.zeros(n_shot_core, np.float32)
        amask[(shot_of[c["gids"]] // NC)] = 1.0

        c["arrays"] = dict(
            xsTV=xsTV, eaTV=eaTV, xs2T=xs2T, ea2T=ea2T, s2T=s2T,
            s3T=s3T, pgT=pgT, invT=invT, amask=amask,
        )
    return cores, meta


# ======================================================
"""Bass/Tile device program (per-core SPMD)."""
import concourse.bass as bass
import concourse.bacc as bacc
import concourse.mybir as mybir
from concourse.tile import TileContext


BF = mybir.dt.bfloat16
FP = mybir.dt.float32
AF = mybir.ActivationFunctionType


def build(meta, num_devices=8):
    NT_V = meta["NT_V"]
    V_pad = meta["V_pad"]
    T_sub = meta["T_sub"]
    E2_slots = meta["E2_slots"]
    GCOLS, G_rows = meta["GCOLS"], meta["G_rows"]
    NSH = meta["n_shot_core"]
    NW = meta["NW"]
    NT_G = G_rows // P

    nc = bacc.Bacc("TRN2", target_bir_lowering=False, debug=False,
                   num_devices=num_devices)

    def inp(name, shape, dt):
        return nc.dram_tensor(name, shape, dt, kind="ExternalInput")

    xsTV_d = inp("xsTV", [SLOT_W, V_pad], BF)
    eaTV_d = inp("eaTV", [SLOT_W, V_pad], BF)
    xs2T_d = inp("xs2T", [SLOT_W, E2_slots], BF)
    ea2T_d = inp("ea2T", [SLOT_W, E2_slots], BF)
    w1s_d = inp("w1s", [P, F1], BF)
    s2T_d = inp("s2T", [P, E2_slots], BF)
    s3T_d = inp("s3T", [P, NT_V, MAXG_TILE], BF)
    pgT_d = inp("pgT", [P, NW], mybir.dt.int32)
    invT_d = inp("invT", [P, NT_G], FP)
    amask_d = inp("amask", [12, NSH], FP)
    ident_d = inp("ident", [P, P], BF)
    wrel2_d = inp("wrel2", [P, F2], BF)
    wroot2_d = inp("wroot2", [P, F2], BF)
    wih0_d = inp("wih0", [3, 2, P, P], BF)
    whh0_d = inp("whh0", [3, P, P], BF)
    wih1_d = inp("wih1", [3, P, P], BF)
    whh1_d = inp("whh1", [3, P, P], BF)
    dec_d = inp("dec", [P, 12], BF)
    out_d = nc.dram_tensor("out", [12, NSH], FP, kind="ExternalOutput")

    emb_d = nc.dram_tensor("emb", [G_rows, F2], FP, kind="Internal")

    with TileContext(nc) as tc:
        with (
            tc.tile_pool(name="const", bufs=1) as cpool,
            tc.tile_pool(name="sb", bufs=3) as pool,
            tc.tile_pool(name="big", bufs=3) as bigp,
            tc.tile_pool(name="psA", bufs=3, space="PSUM") as psA,
            tc.tile_pool(name="psH", bufs=2, space="PSUM") as psH,
            tc.tile_pool(name="psP", bufs=1, space="PSUM") as psP,
            tc.tile_pool(name="psC", bufs=2, space="PSUM") as psC,
        ):
            # ---------------- constants / preloads ----------------
            ident = cpool.tile([P, P], BF, tag="ident")
            nc.sync.dma_start(out=ident[:], in_=ident_d[:])
            w1s = cpool.tile([P, F1], BF, tag="w1s")
            nc.sync.dma_start(out=w1s[:], in_=w1s_d[:])
            wrel2 = cpool.tile([P, F2], BF, tag="wrel2")
            nc.sync.dma_start(out=wrel2[:], in_=wrel2_d[:])
            wroot2 = cpool.tile([P, F2], BF, tag="wroot2")
            nc.sync.dma_start(out=wroot2[:], in_=wroot2_d[:])

            wih0 = []
            for gate in range(3):
                for k in range(2):
                    wt = cpool.tile([P, P], BF, tag=f"wih0_{gate}_{k}")
                    nc.sync.dma_start(out=wt[:], in_=wih0_d[gate, k])
                    wih0.append(wt)

            def load3(dram, nm):
                ts = []
                for i in range(3):
                    wt = cpool.tile([P, P], BF, tag=f"{nm}{i}")
                    nc.sync.dma_start(out=wt[:], in_=dram[i])
                    ts.append(wt)
                return ts

            whh0 = load3(whh0_d, "whh0")
            wih1 = load3(wih1_d, "wih1")
            whh1 = load3(whh1_d, "whh1")
            dec = cpool.tile([P, 12], BF, tag="dec")
            nc.sync.dma_start(out=dec[:], in_=dec_d[:])
            am = cpool.tile([12, NSH], FP, tag="am")
            nc.sync.dma_start(out=am[:], in_=amask_d[:])
            s3all = cpool.tile([P, NT_V, MAXG_TILE], BF, tag="s3all")
            nc.sync.dma_start(out=s3all[:], in_=s3T_d[:])
            pgall = cpool.tile([P, NW], mybir.dt.int32, tag="pgall")
            nc.sync.dma_start(out=pgall[:], in_=pgT_d[:])
            invall = cpool.tile([P, NT_G], FP, tag="invall")
            nc.sync.dma_start(out=invall[:], in_=invT_d[:])

            # zero emb via gpsimd queue so the later indirect scatters
            # (same SWDGE FIFO) are ordered after it without a barrier
            zt = cpool.tile([P, F2], FP, tag="zero")
            nc.gpsimd.memset(zt[:], 0.0)
            for t in range(NT_G):
                nc.gpsimd.dma_start(out=emb_d[t * P:(t + 1) * P, :], in_=zt[:])

            # h1^T of V tiles stays resident for conv2's root term
            # (features on partitions, node columns)
            h1TVall = cpool.tile([P, V_pad], BF, tag="h1TVall")

            # ---------------- conv1 over V tiles ----------------
            n_oct = (NT_V + OCT - 1) // OCT
            for o in range(n_oct):
                t0 = o * OCT
                nt = min(OCT, NT_V - t0)
                cols = slice(t0 * P, (t0 + nt) * P)
                xs_t = pool.tile([SLOT_W, OCT * P], BF, tag="xs1")
                nc.sync.dma_start(out=xs_t[:, :nt * P], in_=xsTV_d[:, cols])
                ea_t = pool.tile([SLOT_W, OCT * P], BF, tag="ea1")
                nc.sync.dma_start(out=ea_t[:, :nt * P], in_=eaTV_d[:, cols])
                msgT = pool.tile([SLOT_W, OCT * P], BF, tag="msg1")
                nc.vector.tensor_mul(out=msgT[:, :nt * P], in0=xs_t[:, :nt * P],
                                     in1=ea_t[:, :nt * P])
                for g0 in range(0, nt, 4):
                    tg = t0 + g0
                    ng = min(4, nt - g0)
                    h1p = psC.tile([P, 4 * P], FP, tag="pC")
                    nc.tensor.matmul(
                        h1p[:, :ng * P], lhsT=w1s[:SLOT_W, :],
                        rhs=msgT[:, g0 * P:(g0 + ng) * P],
                        start=True, stop=True)
                    dst = h1TVall[:, tg * P:(tg + ng) * P]
                    if (o + g0 // 4) % 2 == 0:
                        nc.scalar.activation(dst, h1p[:, :ng * P], AF.Relu)
                    else:
                        nc.vector.tensor_relu(out=dst, in_=h1p[:, :ng * P])

            # ---------------- conv2 + pool (gather-free) ----------------
            GG = 8
            sub_start = np.concatenate([[0], np.cumsum(T_sub)]).astype(int)
            plan = []
            t = 0
            while t < NT_V:
                te = t
                while te < NT_V and sub_start[te + 1] - sub_start[t] <= GG:
                    te += 1
                plan.append((t, te))
                t = te
            pool_ps = None
            for (ta, te) in plan:
                so0 = int(sub_start[ta])
                ns = int(sub_start[te]) - so0
                ecols = slice(so0 * P, (so0 + ns) * P)
                xs2_t = pool.tile([SLOT_W, GG * P], BF, tag="xs2")
                nc.sync.dma_start(out=xs2_t[:, :ns * P], in_=xs2T_d[:, ecols])
                ea2_t = pool.tile([SLOT_W, GG * P], BF, tag="ea2")
                nc.sync.dma_start(out=ea2_t[:, :ns * P], in_=ea2T_d[:, ecols])
                msg2 = pool.tile([SLOT_W, GG * P], BF, tag="msg2")
                nc.vector.tensor_mul(out=msg2[:, :ns * P], in0=xs2_t[:, :ns * P],
                                     in1=ea2_t[:, :ns * P])
                s2g = bigp.tile([P, GG, P], BF, tag="s2g")
                nc.sync.dma_start(
                    out=s2g[:, :ns, :],
                    in_=s2T_d[:, ecols].rearrange("p (s q) -> p s q", q=P))
                # h1 of edge sources, 4 subtiles per PSUM bank
                gts = bigp.tile([P, GG * F1], BF, tag="gts")
                for sb in range(0, ns, 4):
                    nb = min(4, ns - sb)
                    hep = psC.tile([P, 4 * P], FP, tag="pC")
                    for k in range(nb):
                        nc.tensor.matmul(
                            hep[:, k * P:(k + 1) * P],
                            lhsT=msg2[:, (sb + k) * P:(sb + k + 1) * P],
                            rhs=w1s[:SLOT_W, :],
                            start=True, stop=True)
                    dst = gts[:, sb * F1:(sb + nb) * F1]
                    if (sb // 4) % 2 == 0:
                        nc.scalar.activation(dst, hep[:, :nb * P], AF.Relu)
                    else:
                        nc.vector.tensor_relu(out=dst, in_=hep[:, :nb * P])
                for t in range(ta, te):
                    so = int(sub_start[t]) - so0
                    nsub = T_sub[t]
                    agg2T = psA.tile([P, P], FP, tag="pA")
                    for s in range(nsub):
                        nc.tensor.matmul(
                            agg2T[:], lhsT=gts[:, (so + s) * F1:(so + s + 1) * F1],
                            rhs=s2g[:, so + s, :],
                            start=(s == 0), stop=(s == nsub - 1))
                    agg2Ts = pool.tile([P, P], BF, tag="agg2Ts")
                    nc.vector.tensor_copy(out=agg2Ts[:], in_=agg2T[:])
                    h2p = psH.tile([P, F2], FP, tag="pB")
                    nc.tensor.matmul(h2p[:], lhsT=agg2Ts[:], rhs=wrel2[:],
                                     start=True, stop=False)
                    nc.tensor.matmul(h2p[:], lhsT=h1TVall[:, t * P:(t + 1) * P],
                                     rhs=wroot2[:], start=False, stop=True)
                    h2s = pool.tile([P, F2], BF, tag="h2s")
                    if t % 2 == 0:
                        nc.scalar.activation(h2s[:], h2p[:], AF.Relu)
                    else:
                        nc.vector.tensor_relu(out=h2s[:], in_=h2p[:])
                    jj = t % 4
                    if jj == 0:
                        pool_ps = psP.tile([P, F2], FP, tag="pP")
                    nc.tensor.matmul(
                        pool_ps[32 * jj:32 * jj + 32, :], lhsT=s3all[:, t, :],
                        rhs=h2s[:], start=True, stop=True,
                        tile_position=(0, 32 * jj))
                    if jj == 3 or t == NT_V - 1:
                        npart = 32 * (jj + 1)
                        w = t // 4
                        pls = pool.tile([P, F2], FP, tag="pls")
                        nc.vector.tensor_copy(out=pls[:npart, :],
                                              in_=pool_ps[:npart, :])
                        nc.gpsimd.indirect_dma_start(
                            out=emb_d[:, :],
                            out_offset=bass.IndirectOffsetOnAxis(
                                ap=pgall[:npart, w:w + 1], axis=0),
                            in_=pls[:npart, :], in_offset=None,
                            bounds_check=GCOLS, oob_is_err=False)

            tc.strict_bb_all_engine_barrier()

            # ---------------- emb -> embT ----------------
            emball = cpool.tile([P, NT_G, F2], FP, tag="emball")
            nc.sync.dma_start(
                out=emball[:],
                in_=emb_d[:].rearrange("(t p) f -> p t f", p=P))
            embT0 = cpool.tile([P, G_rows], BF, tag="embT0")
            embT1 = cpool.tile([P, G_rows], BF, tag="embT1")
            for t in range(NT_G):
                etb = pool.tile([P, F2], BF, tag="etb")
                nc.vector.tensor_scalar_mul(out=etb[:], in0=emball[:, t, :],
                                            scalar1=invall[:, t:t + 1])
                for half in range(2):
                    tp = psA.tile([P, P], FP, tag="pA")
                    nc.tensor.matmul(tp[:], lhsT=etb[:, half * P:(half + 1) * P],
                                     rhs=ident[:], start=True, stop=True)
                    dst = embT0 if half == 0 else embT1
                    nc.vector.tensor_copy(out=dst[:, t * P:(t + 1) * P], in_=tp[:])

            # ---------------- GRU ----------------
            def batched_gi(xall, wblocks, kt, nm):
                gis = []
                for gate in range(3):
                    gi = cpool.tile([P, GCOLS], FP, tag=f"gi{nm}{gate}")
                    for c0 in range(0, GCOLS, 512):
                        n = min(512, GCOLS - c0)
                        gp = psC.tile([P, 512], FP, tag="pC")
                        for k in range(kt):
                            nc.tensor.matmul(
                                gp[:, :n], lhsT=wblocks[gate * kt + k][:],
                                rhs=xall[k][:, c0:c0 + n],
                                start=(k == 0), stop=(k == kt - 1))
                        nc.vector.tensor_copy(out=gi[:, c0:c0 + n], in_=gp[:, :n])
                    gis.append(gi)
                return gis

            def gru_layer(xall, wih, whh, kt, yout, nm):
                gis = batched_gi(xall, wih, kt, nm)
                h = cpool.tile([P, NSH], BF, tag=f"h_{nm}")
                nc.gpsimd.memset(h[:], 0.0)
                for t in range(TR):
                    ghp = psC.tile([P, 512], FP, tag="pC")
                    for gate in range(3):
                        nc.tensor.matmul(ghp[:, gate * P:(gate + 1) * P],
                                         lhsT=whh[gate][:], rhs=h[:],
                                         start=True, stop=True)

                    def gsl(gate):
                        return gis[gate][:, t::TR][:, :NSH]
                    rs = pool.tile([P, NSH], FP, tag="rs")
                    nc.vector.tensor_add(out=rs[:], in0=gsl(0), in1=ghp[:, 0:P])
                    nc.scalar.activation(rs[:], rs[:], AF.Sigmoid)
                    zs = pool.tile([P, NSH], FP, tag="zs")
                    nc.vector.tensor_add(out=zs[:], in0=gsl(1), in1=ghp[:, P:2 * P])
                    nc.scalar.activation(zs[:], zs[:], AF.Sigmoid)
                    ns_ = pool.tile([P, NSH], FP, tag="ns")
                    nc.vector.tensor_mul(out=ns_[:], in0=rs[:], in1=ghp[:, 2 * P:3 * P])
                    nc.vector.tensor_add(out=ns_[:], in0=ns_[:], in1=gsl(2))
                    nc.scalar.activation(ns_[:], ns_[:], AF.Tanh)
                    hmn = pool.tile([P, NSH], FP, tag="hmn")
                    nc.vector.tensor_sub(out=hmn[:], in0=h[:], in1=ns_[:])
                    nc.vector.tensor_mul(out=hmn[:], in0=hmn[:], in1=zs[:])
                    nc.vector.tensor_add(out=h[:], in0=ns_[:], in1=hmn[:])
                    if yout is not None:
                        nc.vector.tensor_copy(out=yout[:, t::TR][:, :NSH], in_=h[:])
                return h

            y0 = cpool.tile([P, GCOLS], BF, tag="y0")
            gru_layer([embT0, embT1], wih0, whh0, 2, y0, "L0")
            hlast = gru_layer([y0], wih1, whh1, 1, None, "L1")

            lp = psA.tile([P, P], FP, tag="pA")
            nc.tensor.matmul(lp[:12, :NSH], lhsT=dec[:], rhs=hlast[:],
                             start=True, stop=True)
            lo = pool.tile([12, NSH], FP, tag="lo")
            nc.vector.tensor_mul(out=lo[:], in0=lp[:12, :NSH], in1=am[:])
            nc.sync.dma_start(out=out_d[:], in_=lo[:])

    nc.compile()
    return nc


def make_in_map(c, meta, W):
    """Per-core input arrays for run_bass_kernel_spmd."""
    A = c["arrays"]
    bf = lambda a: np.ascontiguousarray(a, dtype=BF16)
    f32 = lambda a: np.ascontiguousarray(a, dtype=np.float32)

    w1s = np.zeros((P, F1), np.float32)
    w1s[0:KSLOT * F_IN] = np.tile(f32(W["c1_wrel"]), (KSLOT, 1))
    w1s[KSLOT * F_IN:SLOT_W] = f32(W["c1_wroot"])
    wih0 = np.stack([np.stack([f32(W["w_ih0"])[g * P:(g + 1) * P, k * P:(k + 1) * P].T
                               for k in range(2)]) for g in range(3)])
    whh0 = np.stack([f32(W["w_hh0"])[g * P:(g + 1) * P, :].T for g in range(3)])
    wih1 = np.stack([f32(W["w_ih1"])[g * P:(g + 1) * P, :].T for g in range(3)])
    whh1 = np.stack([f32(W["w_hh1"])[g * P:(g + 1) * P, :].T for g in range(3)])
    amask = np.broadcast_to(A["amask"][None, :], (12, meta["n_shot_core"]))

    return {
        "xsTV": A["xsTV"],
        "eaTV": A["eaTV"],
        "xs2T": A["xs2T"],
        "ea2T": A["ea2T"],
        "w1s": bf(w1s),
        "s2T": A["s2T"],
        "s3T": A["s3T"],
        "pgT": A["pgT"],
        "invT": A["invT"],
        "amask": f32(amask),
        "ident": bf(np.eye(P, dtype=np.float32)),
        "wrel2": bf(W["c2_wrel"]),
        "wroot2": bf(W["c2_wroot"]),
        "wih0": bf(wih0),
        "whh0": bf(whh0),
        "wih1": bf(wih1),
        "whh1": bf(whh1),
        "dec": bf(W["dec_w"]),
    }


# ------------------------------------------------------------------
_CACHE = {}


def _get_nc(meta):
    key = (meta["NT_V"], meta["E2_slots"], meta["G_rows"],
           tuple(meta["T_sub"]))
    if key not in _CACHE:
        _CACHE[key] = build(meta, num_devices=NC)
    return _CACHE[key]


def kernel(**inputs):
    import sys as _sys
    if "/opt/trn_rl_repo" not in _sys.path:
        _sys.path.insert(0, "/opt/trn_rl_repo")
    from concourse.bass_utils import run_bass_kernel_spmd

    for k in ("c1_b", "c2_b", "b_ih0", "b_hh0", "b_ih1", "b_hh1", "dec_b",
              "empty_emb"):
        assert not np.any(np.asarray(inputs[k])), f"nonzero {k} unsupported"

    cores, meta = prep(inputs)
    W = {k: np.asarray(v, np.float32) for k, v in inputs.items()
         if k not in ("x", "edge_index", "edge_attr", "batch_labels",
                      "label_map", "B")}
    nc = _get_nc(meta)
    in_maps = [make_in_map(c, meta, W) for c in cores]
    res = None
    for attempt in range(6):
        try:
            res = run_bass_kernel_spmd(nc, in_maps, core_ids=list(range(NC)))
            break
        except Exception:
            if attempt == 5:
                raise
    global LAST_RES
    LAST_RES = res
    B = meta["B"]
    out = np.zeros((B, 12), np.float32)
    nsh = meta["n_shot_core"]
    for d in range(NC):
        lg = res.results[d]["out"]          # [12, nsh]
        s = d + NC * np.arange(nsh)
        out[s[s < B]] = lg.T[s < B]
    return out


# revision 28
# speedup vs baseline: 1.2304x; 1.0928x over previous
"""Host-side sharding/prep + Bass device program for nn_BBGRUDecoder.

Host does index manipulation / data layout only; the device kernel does all
model arithmetic.

v4 design:
- conv1 slot arrays carry the root feature as slot KSLOT (weight 1.0) and are
  shipped pre-transposed [SLOT_W, rows] so the conv1 matmul needs no
  on-device transpose and no tree-reduce.
- conv2 does NOT gather h1 rows (SWDGE gather costs ~10ns/row on gpsimd).
  Instead the host lays out each edge's SOURCE-node slot data edge-major
  ([SLOT_W, E2_slots]) and the device recomputes h1 per edge subtile with one
  extra matmul+relu. No gather, no compaction, no DRAM h1 tables.
- conv1 computes h1 only for local V tiles (root term), kept resident in SBUF.
- s2 (edge->dst scatter weights) shipped transposed [128, E2_slots] for wide
  contiguous loads; s3/pool_gid/inv_cnt/emb are single-DMA preloads.
"""
import numpy as np
import ml_dtypes

BF16 = np.dtype(ml_dtypes.bfloat16)
NC = 8
P = 128
KSLOT = 16       # conv1 in-edge slots per node (max in-degree 13)
KSLOT2 = 17      # + root slot
F_IN = 5
SLOT_W = KSLOT2 * F_IN   # 85
F1 = 128
F2 = 256
HID = 128
TR = 10          # rounds per shot
MAXG_TILE = 32   # max graphs per node-tile (pool S3 width)
OCT = 16         # conv1 tiles per input DMA


def _pack_groups(sizes, esizes, cap_items, cap_groups, cap_edges):
    """Greedy-pack consecutive groups (each <=cap_items items) into tiles of
    <=cap_items items, <=cap_groups groups, and <=cap_edges edges (the edge
    cap keeps per-tile conv2 subtile counts uniform across cores)."""
    tiles = []
    i = 0
    n = len(sizes)
    while i < n:
        items = 0
        edges = 0
        g = 0
        while (i + g < n and g < cap_groups
               and items + sizes[i + g] <= cap_items
               and edges + esizes[i + g] <= cap_edges):
            items += sizes[i + g]
            edges += esizes[i + g]
            g += 1
        assert g > 0, (f"group {i} size {sizes[i]}/{esizes[i]} exceeds caps "
                       f"{cap_items}/{cap_edges}")
        tiles.append((i, g, items))
        i += g
    return tiles


def prep(inputs):
    x = np.asarray(inputs["x"], np.float32)
    ei = np.asarray(inputs["edge_index"], np.int64)
    ea = np.asarray(inputs["edge_attr"], np.float32)
    bl = np.asarray(inputs["batch_labels"], np.int64)
    lm = np.asarray(inputs["label_map"], np.int64)
    B = int(inputs["B"])
    NN = x.shape[0]
    src_g, dst_g = ei[0], ei[1]
    shot_of, round_of = lm[:, 0], lm[:, 1]
    n_shot_core = (B + NC - 1) // NC          # 128 shots per core
    GCOLS = n_shot_core * TR                  # 1280 graph-columns per core
    deg = np.bincount(dst_g, minlength=NN)
    assert deg.max() <= KSLOT

    # ---- global conv1 slot data [NN, KSLOT2, F_IN]; slot KSLOT = root ----
    xs_all = np.zeros((NN, KSLOT2, F_IN), np.float32)
    ea_all = np.zeros((NN, KSLOT2, F_IN), np.float32)
    xs_all[:, KSLOT] = x
    ea_all[:, KSLOT] = 1.0
    order = np.argsort(dst_g, kind="stable")
    ds = dst_g[order]
    sl = np.arange(len(ds)) - np.searchsorted(ds, ds)   # slot within dst run
    xs_all[ds, sl] = x[src_g[order]]
    ea_all[ds, sl] = ea[order][:, None]
    xs_flat = xs_all.reshape(NN, SLOT_W).astype(BF16)
    ea_flat = ea_all.reshape(NN, SLOT_W).astype(BF16)

    node_g = bl
    node_core = (shot_of[node_g] % NC).astype(np.int64)

    cores = []
    for d in range(NC):
        V = np.nonzero(node_core == d)[0]          # ascending node ids
        gids, gstart, gcnt = np.unique(node_g[V], return_index=True, return_counts=True)
        s_idx = shot_of[gids] // NC
        gcol = s_idx * TR + round_of[gids]
        # per-graph conv2 edge counts (in-edges of the graph's nodes)
        gedge = np.zeros(len(gids), np.int64)
        np.add.at(gedge, np.searchsorted(gids, node_g[V]), deg[V])
        tiles = _pack_groups(gcnt.tolist(), gedge.tolist(), P, MAXG_TILE, 384)
        NT_V = len(tiles)
        vpos = np.full(NN, -1, np.int64)
        packed_rows = []
        tile_graphs = []
        for (g0, ng, ni) in tiles:
            rows = []
            for k in range(g0, g0 + ng):
                rows.append(V[gstart[k]:gstart[k] + gcnt[k]])
            rows = np.concatenate(rows)
            packed_rows.append(rows)
            tile_graphs.append((gcol[g0:g0 + ng], gcnt[g0:g0 + ng]))
        for t, rows in enumerate(packed_rows):
            vpos[rows] = t * P + np.arange(len(rows))

        E = np.nonzero(node_core[dst_g] == d)[0]
        cores.append(dict(
            d=d, V=V, NT_V=NT_V, packed_rows=packed_rows,
            tile_graphs=tile_graphs, vpos=vpos,
            e_src=src_g[E], e_dst=dst_g[E], e_ea=ea[E],
            gids=gids, gcol=gcol, gcnt=gcnt,
        ))

    # ---- shared static shapes ----
    NT_V = max(c["NT_V"] for c in cores)
    V_pad = NT_V * P

    T_sub = np.zeros(NT_V, np.int64)
    for c in cores:
        for t in range(NT_V):
            if t < c["NT_V"]:
                ne = int(deg[c["packed_rows"][t]].sum())
            else:
                ne = 0
            T_sub[t] = max(T_sub[t], -(-ne // P) if ne else 1)
    E2_slots = int(T_sub.sum()) * P
    NW = -(-NT_V // 4)     # scatter windows (4 tiles each)

    meta = dict(NT_V=NT_V, V_pad=V_pad, T_sub=T_sub.tolist(),
                E2_slots=E2_slots, GCOLS=GCOLS, G_rows=-(-(GCOLS + 1) // P) * P,
                n_shot_core=n_shot_core, B=B, NW=NW)

    # ---- per-core padded arrays ----
    for c in cores:
        vpos = c["vpos"]
        # conv1 V slot data, transposed [SLOT_W, V_pad]
        xsV = np.zeros((V_pad, SLOT_W), BF16)
        eaV = np.zeros((V_pad, SLOT_W), BF16)
        for t, rows in enumerate(c["packed_rows"]):
            xsV[t * P:t * P + len(rows)] = xs_flat[rows]
            eaV[t * P:t * P + len(rows)] = ea_flat[rows]
        xsTV = np.ascontiguousarray(xsV.T)
        eaTV = np.ascontiguousarray(eaV.T)

        # conv2: edge-major src slot data + s2 scatter weights, per tile
        xs2 = np.zeros((E2_slots, SLOT_W), BF16)
        ea2 = np.zeros((E2_slots, SLOT_W), BF16)
        s2 = np.zeros((E2_slots // P, P, P), np.float32)
        st = 0
        for t in range(NT_V):
            nsub = int(T_sub[t])
            if t < c["NT_V"]:
                sel = np.nonzero((vpos[c["e_dst"]] >= t * P) &
                                 (vpos[c["e_dst"]] < t * P + P))[0]
                es, ed, ew = c["e_src"][sel], c["e_dst"][sel], c["e_ea"][sel]
                ne = len(es)
                xs2[st * P:st * P + ne] = xs_flat[es]
                ea2[st * P:st * P + ne] = ea_flat[es]
                loc = vpos[ed] - t * P
                s2[st + np.arange(ne) // P, np.arange(ne) % P, loc] = ew
            st += nsub
        assert st * P == E2_slots
        xs2T = np.ascontiguousarray(xs2.T)
        ea2T = np.ascontiguousarray(ea2.T)
        s2T = np.ascontiguousarray(
            s2.transpose(1, 0, 2).reshape(P, E2_slots).astype(BF16))

        # pool S3 / graph ids / inv counts
        s3 = np.zeros((NT_V, P, MAXG_TILE), np.float32)
        pool_gid = np.full((NT_V, MAXG_TILE), meta["GCOLS"] + 100, np.int64)
        for t in range(c["NT_V"]):
            gcols, gcnts = c["tile_graphs"][t]
            off = 0
            for j, (gc, n) in enumerate(zip(gcols, gcnts)):
                s3[t, off:off + n, j] = 1
                pool_gid[t, j] = gc
                off += n
        s3T = np.ascontiguousarray(s3.transpose(1, 0, 2).astype(BF16))
        pg_pad = np.full((NW * 4, MAXG_TILE), meta["GCOLS"] + 100, np.int64)
        pg_pad[:NT_V] = pool_gid
        pgT = np.ascontiguousarray(pg_pad.reshape(NW, P).T.astype(np.int32))
        inv_cnt = np.zeros(meta["G_rows"], np.float32)
        inv_cnt[c["gcol"]] = 1.0 / np.maximum(c["gcnt"], 1)
        invT = np.ascontiguousarray(inv_cnt.reshape(-1, P).T)

        amask = np.zeros(n_shot_core, np.float32)
        amask[(shot_of[c["gids"]] // NC)] = 1.0

        c["arrays"] = dict(
            xsTV=xsTV, eaTV=eaTV, xs2T=xs2T, ea2T=ea2T, s2T=s2T,
            s3T=s3T, pgT=pgT, invT=invT, amask=amask,
        )
    return cores, meta


# ======================================================
"""Bass/Tile device program (per-core SPMD)."""
import concourse.bass as bass
import concourse.bacc as bacc
import concourse.mybir as mybir
from concourse.tile import TileContext


BF = mybir.dt.bfloat16
FP = mybir.dt.float32
AF = mybir.ActivationFunctionType


def build(meta, num_devices=8):
    NT_V = meta["NT_V"]
    V_pad = meta["V_pad"]
    T_sub = meta["T_sub"]
    E2_slots = meta["E2_slots"]
    GCOLS, G_rows = meta["GCOLS"], meta["G_rows"]
    NSH = meta["n_shot_core"]
    NW = meta["NW"]
    NT_G = G_rows // P

    nc = bacc.Bacc("TRN2", target_bir_lowering=False, debug=False,
                   num_devices=num_devices)

    def inp(name, shape, dt):
        return nc.dram_tensor(name, shape, dt, kind="ExternalInput")

    xsTV_d = inp("xsTV", [SLOT_W, V_pad], BF)
    eaTV_d = inp("eaTV", [SLOT_W, V_pad], BF)
    xs2T_d = inp("xs2T", [SLOT_W, E2_slots], BF)
    ea2T_d = inp("ea2T", [SLOT_W, E2_slots], BF)
    w1s_d = inp("w1s", [P, F1], BF)
    s2T_d = inp("s2T", [P, E2_slots], BF)
    s3T_d = inp("s3T", [P, NT_V, MAXG_TILE], BF)
    pgT_d = inp("pgT", [P, NW], mybir.dt.int32)
    invT_d = inp("invT", [P, NT_G], FP)
    amask_d = inp("amask", [12, NSH], FP)
    ident_d = inp("ident", [P, P], BF)
    wrel2_d = inp("wrel2", [P, F2], BF)
    wroot2_d = inp("wroot2", [P, F2], BF)
    wih0_d = inp("wih0", [3, 2, P, P], BF)
    whh0_d = inp("whh0", [3, P, P], BF)
    wih1_d = inp("wih1", [3, P, P], BF)
    whh1_d = inp("whh1", [3, P, P], BF)
    dec_d = inp("dec", [P, 12], BF)
    out_d = nc.dram_tensor("out", [12, NSH], FP, kind="ExternalOutput")

    emb_d = nc.dram_tensor("emb", [G_rows, F2], FP, kind="Internal")

    with TileContext(nc) as tc:
        with (
            tc.tile_pool(name="const", bufs=1) as cpool,
            tc.tile_pool(name="sb", bufs=3) as pool,
            tc.tile_pool(name="big", bufs=3) as bigp,
            tc.tile_pool(name="psA", bufs=3, space="PSUM") as psA,
            tc.tile_pool(name="psH", bufs=2, space="PSUM") as psH,
            tc.tile_pool(name="psP", bufs=1, space="PSUM") as psP,
            tc.tile_pool(name="psC", bufs=2, space="PSUM") as psC,
        ):
            # ---------------- constants / preloads ----------------
            ident = cpool.tile([P, P], BF, tag="ident")
            nc.sync.dma_start(out=ident[:], in_=ident_d[:])
            w1s = cpool.tile([P, F1], BF, tag="w1s")
            nc.sync.dma_start(out=w1s[:], in_=w1s_d[:])
            wrel2 = cpool.tile([P, F2], BF, tag="wrel2")
            nc.sync.dma_start(out=wrel2[:], in_=wrel2_d[:])
            wroot2 = cpool.tile([P, F2], BF, tag="wroot2")
            nc.sync.dma_start(out=wroot2[:], in_=wroot2_d[:])

            wih0 = []
            for gate in range(3):
                for k in range(2):
                    wt = cpool.tile([P, P], BF, tag=f"wih0_{gate}_{k}")
                    nc.sync.dma_start(out=wt[:], in_=wih0_d[gate, k])
                    wih0.append(wt)

            def load3(dram, nm):
                ts = []
                for i in range(3):
                    wt = cpool.tile([P, P], BF, tag=f"{nm}{i}")
                    nc.sync.dma_start(out=wt[:], in_=dram[i])
                    ts.append(wt)
                return ts

            whh0 = load3(whh0_d, "whh0")
            wih1 = load3(wih1_d, "wih1")
            whh1 = load3(whh1_d, "whh1")
            dec = cpool.tile([P, 12], BF, tag="dec")
            nc.sync.dma_start(out=dec[:], in_=dec_d[:])
            am = cpool.tile([12, NSH], FP, tag="am")
            nc.sync.dma_start(out=am[:], in_=amask_d[:])
            s3all = cpool.tile([P, NT_V, MAXG_TILE], BF, tag="s3all")
            nc.sync.dma_start(out=s3all[:], in_=s3T_d[:])
            pgall = cpool.tile([P, NW], mybir.dt.int32, tag="pgall")
            nc.sync.dma_start(out=pgall[:], in_=pgT_d[:])
            invall = cpool.tile([P, NT_G], FP, tag="invall")
            nc.sync.dma_start(out=invall[:], in_=invT_d[:])

            # zero emb via gpsimd queue so the later indirect scatters
            # (same SWDGE FIFO) are ordered after it without a barrier
            zt = cpool.tile([P, F2], FP, tag="zero")
            nc.gpsimd.memset(zt[:], 0.0)
            for t in range(NT_G):
                nc.gpsimd.dma_start(out=emb_d[t * P:(t + 1) * P, :], in_=zt[:])

            # h1^T of V tiles stays resident for conv2's root term
            # (features on partitions, node columns)
            h1TVall = cpool.tile([P, V_pad], BF, tag="h1TVall")

            # ---------------- conv1 over V tiles ----------------
            n_oct = (NT_V + OCT - 1) // OCT
            for o in range(n_oct):
                t0 = o * OCT
                nt = min(OCT, NT_V - t0)
                cols = slice(t0 * P, (t0 + nt) * P)
                xs_t = pool.tile([SLOT_W, OCT * P], BF, tag="xs1")
                nc.sync.dma_start(out=xs_t[:, :nt * P], in_=xsTV_d[:, cols])
                ea_t = pool.tile([SLOT_W, OCT * P], BF, tag="ea1")
                nc.sync.dma_start(out=ea_t[:, :nt * P], in_=eaTV_d[:, cols])
                msgT = pool.tile([SLOT_W, OCT * P], BF, tag="msg1")
                nc.vector.tensor_mul(out=msgT[:, :nt * P], in0=xs_t[:, :nt * P],
                                     in1=ea_t[:, :nt * P])
                for g0 in range(0, nt, 4):
                    tg = t0 + g0
                    ng = min(4, nt - g0)
                    h1p = psC.tile([P, 4 * P], FP, tag="pC")
                    nc.tensor.matmul(
                        h1p[:, :ng * P], lhsT=w1s[:SLOT_W, :],
                        rhs=msgT[:, g0 * P:(g0 + ng) * P],
                        start=True, stop=True)
                    dst = h1TVall[:, tg * P:(tg + ng) * P]
                    if (o + g0 // 4) % 2 == 0:
                        nc.scalar.activation(dst, h1p[:, :ng * P], AF.Relu)
                    else:
                        nc.vector.tensor_relu(out=dst, in_=h1p[:, :ng * P])

            # ---------------- conv2 + pool (gather-free) ----------------
            GG = 8
            sub_start = np.concatenate([[0], np.cumsum(T_sub)]).astype(int)
            plan = []
            t = 0
            while t < NT_V:
                te = t
                while te < NT_V and sub_start[te + 1] - sub_start[t] <= GG:
                    te += 1
                plan.append((t, te))
                t = te
            pool_ps = None
            for (ta, te) in plan:
                so0 = int(sub_start[ta])
                ns = int(sub_start[te]) - so0
                ecols = slice(so0 * P, (so0 + ns) * P)
                xs2_t = pool.tile([SLOT_W, GG * P], BF, tag="xs2")
                nc.sync.dma_start(out=xs2_t[:, :ns * P], in_=xs2T_d[:, ecols])
                ea2_t = pool.tile([SLOT_W, GG * P], BF, tag="ea2")
                nc.sync.dma_start(out=ea2_t[:, :ns * P], in_=ea2T_d[:, ecols])
                msg2 = pool.tile([SLOT_W, GG * P], BF, tag="msg2")
                nc.vector.tensor_mul(out=msg2[:, :ns * P], in0=xs2_t[:, :ns * P],
                                     in1=ea2_t[:, :ns * P])
                s2g = bigp.tile([P, GG, P], BF, tag="s2g")
                nc.sync.dma_start(
                    out=s2g[:, :ns, :],
                    in_=s2T_d[:, ecols].rearrange("p (s q) -> p s q", q=P))
                # h1 of edge sources, 4 subtiles per PSUM bank
                gts = bigp.tile([P, GG * F1], BF, tag="gts")
                for sb in range(0, ns, 4):
                    nb = min(4, ns - sb)
                    hep = psC.tile([P, 4 * P], FP, tag="pC")
                    for k in range(nb):
                        nc.tensor.matmul(
                            hep[:, k * P:(k + 1) * P],
                            lhsT=msg2[:, (sb + k) * P:(sb + k + 1) * P],
                            rhs=w1s[:SLOT_W, :],
                            start=True, stop=True)
                    dst = gts[:, sb * F1:(sb + nb) * F1]
                    if (sb // 4) % 2 == 0:
                        nc.scalar.activation(dst, hep[:, :nb * P], AF.Relu)
                    else:
                        nc.vector.tensor_relu(out=dst, in_=hep[:, :nb * P])
                for t in range(ta, te):
                    so = int(sub_start[t]) - so0
                    nsub = T_sub[t]
                    agg2T = psA.tile([P, P], FP, tag="pA")
                    for s in range(nsub):
                        nc.tensor.matmul(
                            agg2T[:], lhsT=gts[:, (so + s) * F1:(so + s + 1) * F1],
                            rhs=s2g[:, so + s, :],
                            start=(s == 0), stop=(s == nsub - 1))
                    agg2Ts = pool.tile([P, P], BF, tag="agg2Ts")
                    nc.vector.tensor_copy(out=agg2Ts[:], in_=agg2T[:])
                    h2p = psH.tile([P, F2], FP, tag="pB")
                    nc.tensor.matmul(h2p[:], lhsT=agg2Ts[:], rhs=wrel2[:],
                                     start=True, stop=False)
                    nc.tensor.matmul(h2p[:], lhsT=h1TVall[:, t * P:(t + 1) * P],
                                     rhs=wroot2[:], start=False, stop=True)
                    h2s = pool.tile([P, F2], BF, tag="h2s")
                    if t % 2 == 0:
                        nc.scalar.activation(h2s[:], h2p[:], AF.Relu)
                    else:
                        nc.vector.tensor_relu(out=h2s[:], in_=h2p[:])
                    jj = t % 4
                    if jj == 0:
                        pool_ps = psP.tile([P, F2], FP, tag="pP")
                    nc.tensor.matmul(
                        pool_ps[32 * jj:32 * jj + 32, :], lhsT=s3all[:, t, :],
                        rhs=h2s[:], start=True, stop=True,
                        tile_position=(0, 32 * jj))
                    if jj == 3 or t == NT_V - 1:
                        npart = 32 * (jj + 1)
                        w = t // 4
                        pls = pool.tile([P, F2], FP, tag="pls")
                        nc.vector.tensor_copy(out=pls[:npart, :],
                                              in_=pool_ps[:npart, :])
                        nc.gpsimd.indirect_dma_start(
                            out=emb_d[:, :],
                            out_offset=bass.IndirectOffsetOnAxis(
                                ap=pgall[:npart, w:w + 1], axis=0),
                            in_=pls[:npart, :], in_offset=None,
                            bounds_check=GCOLS, oob_is_err=False)

            tc.strict_bb_all_engine_barrier()

            # ---------------- emb -> embT ----------------
            emball = cpool.tile([P, NT_G, F2], FP, tag="emball")
            nc.sync.dma_start(
                out=emball[:],
                in_=emb_d[:].rearrange("(t p) f -> p t f", p=P))
            embT0 = cpool.tile([P, G_rows], BF, tag="embT0")
            embT1 = cpool.tile([P, G_rows], BF, tag="embT1")
            for t in range(NT_G):
                etb = pool.tile([P, F2], BF, tag="etb")
                nc.vector.tensor_scalar_mul(out=etb[:], in0=emball[:, t, :],
                                            scalar1=invall[:, t:t + 1])
                for half in range(2):
                    tp = psA.tile([P, P], FP, tag="pA")
                    nc.tensor.matmul(tp[:], lhsT=etb[:, half * P:(half + 1) * P],
                                     rhs=ident[:], start=True, stop=True)
                    dst = embT0 if half == 0 else embT1
                    nc.vector.tensor_copy(out=dst[:, t * P:(t + 1) * P], in_=tp[:])

            # ---------------- GRU ----------------
            def batched_gi(xall, wblocks, kt, nm):
                gis = []
                for gate in range(3):
                    gi = cpool.tile([P, GCOLS], FP, tag=f"gi{nm}{gate}")
                    for c0 in range(0, GCOLS, 512):
                        n = min(512, GCOLS - c0)
                        gp = psC.tile([P, 512], FP, tag="pC")
                        for k in range(kt):
                            nc.tensor.matmul(
                                gp[:, :n], lhsT=wblocks[gate * kt + k][:],
                                rhs=xall[k][:, c0:c0 + n],
                                start=(k == 0), stop=(k == kt - 1))
                        nc.vector.tensor_copy(out=gi[:, c0:c0 + n], in_=gp[:, :n])
                    gis.append(gi)
                return gis

            def gru_layer(xall, wih, whh, kt, yout, nm):
                gis = batched_gi(xall, wih, kt, nm)
                h = cpool.tile([P, NSH], BF, tag=f"h_{nm}")
                nc.gpsimd.memset(h[:], 0.0)
                for t in range(TR):
                    ghp = psC.tile([P, 512], FP, tag="pC")
                    for gate in range(3):
                        nc.tensor.matmul(ghp[:, gate * P:(gate + 1) * P],
                                         lhsT=whh[gate][:], rhs=h[:],
                                         start=True, stop=True)

                    def gsl(gate):
                        return gis[gate][:, t::TR][:, :NSH]
                    rs = pool.tile([P, NSH], FP, tag="rs")
                    nc.vector.tensor_add(out=rs[:], in0=gsl(0), in1=ghp[:, 0:P])
                    nc.scalar.activation(rs[:], rs[:], AF.Sigmoid)
                    zs = pool.tile([P, NSH], FP, tag="zs")
                    nc.vector.tensor_add(out=zs[:], in0=gsl(1), in1=ghp[:, P:2 * P])
                    nc.scalar.activation(zs[:], zs[:], AF.Sigmoid)
                    ns_ = pool.tile([P, NSH], FP, tag="ns")
                    nc.vector.tensor_mul(out=ns_[:], in0=rs[:], in1=ghp[:, 2 * P:3 * P])
                    nc.vector.tensor_add(out=ns_[:], in0=ns_[:], in1=gsl(2))
                    nc.scalar.activation(ns_[:], ns_[:], AF.Tanh)
                    hmn = pool.tile([P, NSH], FP, tag="hmn")
                    nc.vector.tensor_sub(out=hmn[:], in0=h[:], in1=ns_[:])
                    nc.vector.tensor_mul(out=hmn[:], in0=hmn[:], in1=zs[:])
                    nc.vector.tensor_add(out=h[:], in0=ns_[:], in1=hmn[:])
                    if yout is not None:
                        nc.vector.tensor_copy(out=yout[:, t::TR][:, :NSH], in_=h[:])
                return h

            y0 = cpool.tile([P, GCOLS], BF, tag="y0")
            gru_layer([embT0, embT1], wih0, whh0, 2, y0, "L0")
            hlast = gru_layer([y0], wih1, whh1, 1, None, "L1")

            lp = psA.tile([P, P], FP, tag="pA")
            nc.tensor.matmul(lp[:12, :NSH], lhsT=dec[:], rhs=hlast[:],
                             start=True, stop=True)
            lo = pool.tile([12, NSH], FP, tag="lo")
            nc.vector.tensor_mul(out=lo[:], in0=lp[:12, :NSH], in1=am[:])
            nc.sync.dma_start(out=out_d[:], in_=lo[:])

    nc.compile()
    return nc


def make_in_map(c, meta, W):
    """Per-core input arrays for run_bass_kernel_spmd."""
    A = c["arrays"]
    bf = lambda a: np.ascontiguousarray(a, dtype=BF16)
    f32 = lambda a: np.ascontiguousarray(a, dtype=np.float32)

    w1s = np.zeros((P, F1), np.float32)
    w1s[0:KSLOT * F_IN] = np.tile(f32(W["c1_wrel"]), (KSLOT, 1))
    w1s[KSLOT * F_IN:SLOT_W] = f32(W["c1_wroot"])
    wih0 = np.stack([np.stack([f32(W["w_ih0"])[g * P:(g + 1) * P, k * P:(k + 1) * P].T
                               for k in range(2)]) for g in range(3)])
    whh0 = np.stack([f32(W["w_hh0"])[g * P:(g + 1) * P, :].T for g in range(3)])
    wih1 = np.stack([f32(W["w_ih1"])[g * P:(g + 1) * P, :].T for g in range(3)])
    whh1 = np.stack([f32(W["w_hh1"])[g * P:(g + 1) * P, :].T for g in range(3)])
    amask = np.broadcast_to(A["amask"][None, :], (12, meta["n_shot_core"]))

    return {
        "xsTV": A["xsTV"],
        "eaTV": A["eaTV"],
        "xs2T": A["xs2T"],
        "ea2T": A["ea2T"],
        "w1s": bf(w1s),
        "s2T": A["s2T"],
        "s3T": A["s3T"],
        "pgT": A["pgT"],
        "invT": A["invT"],
        "amask": f32(amask),
        "ident": bf(np.eye(P, dtype=np.float32)),
        "wrel2": bf(W["c2_wrel"]),
        "wroot2": bf(W["c2_wroot"]),
        "wih0": bf(wih0),
        "whh0": bf(whh0),
        "wih1": bf(wih1),
        "whh1": bf(whh1),
        "dec": bf(W["dec_w"]),
    }


# ------------------------------------------------------------------
_CACHE = {}


def _get_nc(meta):
    key = (meta["NT_V"], meta["E2_slots"], meta["G_rows"],
           tuple(meta["T_sub"]))
    if key not in _CACHE:
        _CACHE[key] = build(meta, num_devices=NC)
    return _CACHE[key]


def kernel(**inputs):
    import sys as _sys
    if "/opt/trn_rl_repo" not in _sys.path:
        _sys.path.insert(0, "/opt/trn_rl_repo")
    from concourse.bass_utils import run_bass_kernel_spmd

    for k in ("c1_b", "c2_b", "b_ih0", "b_hh0", "b_ih1", "b_hh1", "dec_b",
              "empty_emb"):
        assert not np.any(np.asarray(inputs[k])), f"nonzero {k} unsupported"

    cores, meta = prep(inputs)
    W = {k: np.asarray(v, np.float32) for k, v in inputs.items()
         if k not in ("x", "edge_index", "edge_attr", "batch_labels",
                      "label_map", "B")}
    nc = _get_nc(meta)
    in_maps = [make_in_map(c, meta, W) for c in cores]
    res = None
    for attempt in range(6):
        try:
            res = run_bass_kernel_spmd(nc, in_maps, core_ids=list(range(NC)))
            break
        except Exception:
            if attempt == 5:
                raise
    global LAST_RES
    LAST_RES = res
    B = meta["B"]
    out = np.zeros((B, 12), np.float32)
    nsh = meta["n_shot_core"]
    for d in range(NC):
        lg = res.results[d]["out"]          # [12, nsh]
        s = d + NC * np.arange(nsh)
        out[s[s < B]] = lg.T[s < B]
    return out
